# revision 1
# baseline (speedup 1.0000x reference)
"""DiffusionNet forward on 8 Trainium2 NeuronCores.

Strategy
--------
B=4 samples, 2 cores per sample, each core owns half the mesh nodes
(20000, zero-padded to 20480).  All cross-node coupling flows through the
K=128 spectral bottleneck:

  * SpMM is eliminated on-device: gX = G @ x_diffuse = (G @ evecs) @ S with
    S = coefs * x_spec, so host precomputes GXe = G @ evecs once per sample
    (exact associativity; measured 4e-7 rel err).
  * Per block: partial x_spec^T = sum_n x[n,:]^T evm[n,:] over owned nodes
    (PE accumulation), pairwise AllReduce of the [C,K] partial (64KB), then a
    fused channel-major sweep over node chunks computes x_diffuse, gX, gY,
    Breal, Bimag, grad_feat, the MLP and the residual without touching HBM
    for intermediates.

Layouts: per-node tensors live channel-major ([C, n]) in SBUF; x carried in
fp32 (+ a bf16 shadow for matmul operands), streamed operands in bf16.
"""

import sys
import numpy as np
import ml_dtypes

for _p in ("/opt/trn_rl_repo", "/root/.axon_site/_ro/trn_rl_repo"):
    if _p not in sys.path:
        sys.path.append(_p)

import concourse.bass as bass
import concourse.bacc as bacc
import concourse.tile as tile
import concourse.mybir as mybir
from concourse.bass_utils import run_bass_kernel_spmd
from concourse.masks import make_identity

BF = mybir.dt.bfloat16
F32 = mybir.dt.float32
F32R = mybir.dt.float32r
AF = mybir.ActivationFunctionType
ALU = mybir.AluOpType

B, N, E, K = 4, 40000, 240000, 128
C = 128
NB = 4          # diffusion blocks
NCORES = 8
NH = N // 2     # nodes per core (half sample)
CH = 512        # node chunk (matmul free dim)
NHP = 20480     # padded nodes per core: 40 chunks * 512 = 160 tiles * 128
NCH = NHP // CH
NT = NHP // 128
PAIRS = [[0, 1], [2, 3], [4, 5], [6, 7]]

bf16 = ml_dtypes.bfloat16


# ----------------------------------------------------------------- host side

def _spmm_mat(rows, cols, vals, M):
    """(COO [N,N] with given pattern) @ M, dense M [N,k]. Pure numpy."""
    out = np.zeros((N, M.shape[1]), np.float32)
    perm = np.argsort(rows, kind="stable")
    contrib = (vals[:, None] * M[cols]).astype(np.float32)[perm]
    rs = rows[perm]
    uniq, starts = np.unique(rs, return_index=True)
    out[uniq] = np.add.reduceat(contrib, starts, axis=0)
    return out


def host_prep(inputs, nhp=NHP, nb=NB):
    """Build the 8 per-core input dicts."""
    x_in = np.asarray(inputs["x_in"], np.float32)
    mass = np.asarray(inputs["mass"], np.float32)
    evals = np.asarray(inputs["evals"], np.float32)
    evecs = np.asarray(inputs["evecs"], np.float32)
    rows = np.asarray(inputs["rows"])
    cols = np.asarray(inputs["cols"])
    gX_vals = np.asarray(inputs["gradX_vals"], np.float32)
    gY_vals = np.asarray(inputs["gradY_vals"], np.float32)
    w_first = np.asarray(inputs["w_first"], np.float32)
    b_first = np.asarray(inputs["b_first"], np.float32)
    diff_time = np.asarray(inputs["diff_time"], np.float32)
    A_re = np.asarray(inputs["A_re"], np.float32)
    A_im = np.asarray(inputs["A_im"], np.float32)
    mlp_w0 = np.asarray(inputs["mlp_w0"], np.float32)
    w1 = np.asarray(inputs["mlp_w1"], np.float32)
    w2 = np.asarray(inputs["mlp_w2"], np.float32)
    b0 = np.asarray(inputs["mlp_b0"], np.float32)
    b1 = np.asarray(inputs["mlp_b1"], np.float32)
    b2 = np.asarray(inputs["mlp_b2"], np.float32)
    w_last = np.asarray(inputs["w_last"], np.float32)
    b_last = np.asarray(inputs["b_last"], np.float32)

    nh = NH

    shared = dict(
        Are=A_re[:nb],
        Aim=A_im[:nb],
        w0af=np.ascontiguousarray(mlp_w0[:nb, 0:C]),
        w0bf=np.ascontiguousarray(mlp_w0[:nb, C:2 * C]),
        w0c=mlp_w0[:nb, 2 * C:3 * C].astype(bf16),
        w1=w1[:nb].astype(bf16),
        w2=w2[:nb].astype(bf16),
        b0=b0[:nb].reshape(nb, C, 1),
        b1=b1[:nb].reshape(nb, C, 1),
        b2=b2[:nb].reshape(nb, C, 1),
        wlastf=w_last,
        blast=b_last.reshape(3, 1),
    )

    in_maps = []
    for b in range(B):
        ev = evecs[b]
        evm_full = ev * mass[b][:, None]
        GXe = _spmm_mat(rows, cols, gX_vals[b], ev)
        GYe = _spmm_mat(rows, cols, gY_vals[b], ev)
        x0_full = x_in[b] @ w_first + b_first
        # coefsT[i][c,k] = exp(-evals[k] * diff_time[i][c])
        coefsT = np.exp(-evals[b][None, None, :]
                        * diff_time[:nb, :, None]).astype(np.float32)
        for h in range(2):
            sl = slice(h * nh, (h + 1) * nh)

            def padT(M):  # [nh, K] -> [K, nhp]
                out = np.zeros((M.shape[1], nhp), np.float32)
                out[:, :nh] = M[sl].T
                return out

            evmP = np.zeros((nhp, K), np.float32)
            evmP[:nh] = evm_full[sl]
            evm4 = evmP.reshape(nhp // 512, 4, 128, K).transpose(0, 2, 1, 3) \
                       .reshape(nhp // 512, 128, 512)
            x0T = padT(x0_full)
            in_maps.append(dict(
                evm4=evm4.astype(bf16),
                evT=padT(ev).astype(bf16),
                gxT=padT(GXe).astype(bf16),
                gyT=padT(GYe).astype(bf16),
                x0T=x0T,
                coefsT=coefsT,
                **shared,
            ))
    return in_maps


# --------------------------------------------------------------- device side

def build_nc(nb=NB, nch=NCH, ncores=NCORES, collective=True):
    nhp = nch * CH
    nt = nhp // 128
    nc = bacc.Bacc("TRN2", target_bir_lowering=False, debug=False,
                   enable_asserts=True, num_devices=ncores)

    evm4 = nc.dram_tensor("evm4", [nch, 128, 512], BF, kind="ExternalInput")
    evT = nc.dram_tensor("evT", [K, nhp], BF, kind="ExternalInput")
    gxT = nc.dram_tensor("gxT", [K, nhp], BF, kind="ExternalInput")
    gyT = nc.dram_tensor("gyT", [K, nhp], BF, kind="ExternalInput")
    x0T = nc.dram_tensor("x0T", [C, nhp], F32, kind="ExternalInput")
    coefsT = nc.dram_tensor("coefsT", [nb, C, K], F32, kind="ExternalInput")
    Are = nc.dram_tensor("Are", [nb, C, C], F32, kind="ExternalInput")
    Aim = nc.dram_tensor("Aim", [nb, C, C], F32, kind="ExternalInput")
    w0af = nc.dram_tensor("w0af", [nb, C, C], F32, kind="ExternalInput")
    w0bf = nc.dram_tensor("w0bf", [nb, C, C], F32, kind="ExternalInput")
    w0c = nc.dram_tensor("w0c", [nb, C, C], BF, kind="ExternalInput")
    w1 = nc.dram_tensor("w1", [nb, C, C], BF, kind="ExternalInput")
    w2 = nc.dram_tensor("w2", [nb, C, C], BF, kind="ExternalInput")
    b0 = nc.dram_tensor("b0", [nb, C, 1], F32, kind="ExternalInput")
    b1 = nc.dram_tensor("b1", [nb, C, 1], F32, kind="ExternalInput")
    b2 = nc.dram_tensor("b2", [nb, C, 1], F32, kind="ExternalInput")
    wlastf = nc.dram_tensor("wlastf", [C, 3], F32, kind="ExternalInput")
    blast = nc.dram_tensor("blast", [3, 1], F32, kind="ExternalInput")
    yT = nc.dram_tensor("yT", [3, nhp], F32, kind="ExternalOutput")

    with tile.TileContext(nc) as tc:
        with (
            tc.tile_pool(name="consts", bufs=1) as consts,
            tc.tile_pool(name="xpool", bufs=1) as xpool,
            tc.tile_pool(name="stream", bufs=4) as stream,
            tc.tile_pool(name="csb", bufs=3) as csb,
            tc.tile_pool(name="smalls", bufs=2) as smalls,
            tc.tile_pool(name="mm_ps", bufs=7, space="PSUM") as mm_ps,
            tc.tile_pool(name="small_ps", bufs=1, space="PSUM") as small_ps,
            tc.tile_pool(name="dram", bufs=2, space="DRAM") as dram,
        ):
            ident_bf = consts.tile([128, 128], BF, tag="identb")
            make_identity(nc, ident_bf[:])
            ident_f = consts.tile([128, 128], F32, tag="identf")
            make_identity(nc, ident_f[:])

            def cload(src, shape, dt, tag):
                t = consts.tile(shape, dt, tag=tag)
                nc.sync.dma_start(t[:], src)
                return t

            Are_s = [cload(Are[i], [C, C], F32, f"Are{i}") for i in range(nb)]
            Aim_s = [cload(Aim[i], [C, C], F32, f"Aim{i}") for i in range(nb)]
            coefsT_s = [cload(coefsT[i], [C, K], F32, f"cf{i}") for i in range(nb)]
            w0af_s = [cload(w0af[i], [C, C], F32, f"w0af{i}") for i in range(nb)]
            w0bf_s = [cload(w0bf[i], [C, C], F32, f"w0bf{i}") for i in range(nb)]
            w0c_s = [cload(w0c[i], [C, C], BF, f"w0c{i}") for i in range(nb)]
            w1_s = [cload(w1[i], [C, C], BF, f"w1{i}") for i in range(nb)]
            w2_s = [cload(w2[i], [C, C], BF, f"w2{i}") for i in range(nb)]
            b0_s = [cload(b0[i], [C, 1], F32, f"b0{i}") for i in range(nb)]
            b1_s = [cload(b1[i], [C, 1], F32, f"b1{i}") for i in range(nb)]
            b2_s = [cload(b2[i], [C, 1], F32, f"b2{i}") for i in range(nb)]
            wlastf_s = cload(wlastf[:], [C, 3], F32, "wlast")
            blast_s = cload(blast[:], [3, 1], F32, "blast")

            # fp32r copies of the weights used in fp32r matmuls against x
            w0a_r = []
            for i in range(nb):
                t = consts.tile([C, C], F32R, tag=f"w0ar{i}")
                nc.vector.tensor_copy(t[:], w0af_s[i][:])
                w0a_r.append(t)
            wlast_r = consts.tile([C, 3], F32R, tag="wlastr")
            nc.vector.tensor_copy(wlast_r[:], wlastf_s[:])

            xs = []
            for cI in range(nch):
                sl = bass.ts(cI, CH)
                xtmp = stream.tile([C, CH], F32, tag="x0tmp")
                nc.sync.dma_start(xtmp[:], x0T[:, sl])
                xt = xpool.tile([C, CH], F32R, tag=f"x{cI}")
                nc.vector.tensor_copy(xt[:], xtmp[:])
                xs.append(xt)

            for i in range(nb):
                # ---- forward spectral transform: x_spec^T = sum x^T evm ----
                xspec_ps = small_ps.tile([C, K], F32, tag="sps")
                ebuf = None
                for t in range(nt):
                    cI, f = divmod(t, 4)
                    if f == 0:
                        ebuf = stream.tile([128, 512], BF, tag="evm")
                        nc.sync.dma_start(ebuf[:], evm4[cI])
                    tp = mm_ps.tile([128, 128], F32, tag="mm")
                    nc.tensor.transpose(
                        tp[:], xs[cI][:, f * 128:(f + 1) * 128].bitcast(F32),
                        ident_f[:])
                    xt = csb.tile([128, 128], BF, tag="xt")
                    nc.vector.tensor_copy(xt[:], tp[:])
                    nc.tensor.matmul(xspec_ps[:], xt[:],
                                     ebuf[:, f * 128:(f + 1) * 128],
                                     start=(t == 0), stop=(t == nt - 1))

                # coefs multiply commutes with the pairwise sum -> do it
                # before the AllReduce (off the post-collective critical path)
                STf_p = smalls.tile([C, K], F32, tag="xsp")
                nc.vector.tensor_mul(STf_p[:], xspec_ps[:], coefsT_s[i][:])
                if collective:
                    cc_in = dram.tile([C, K], F32, tag="ccin")
                    cc_out = dram.tile([C, K], F32, tag="ccout")
                    nc.sync.dma_start(cc_in[:], STf_p[:])
                    nc.gpsimd.collective_compute(
                        "AllReduce", ALU.add,
                        replica_groups=PAIRS[:ncores // 2],
                        ins=[cc_in.opt()], outs=[cc_out.opt()])
                    STf = smalls.tile([C, K], F32, tag="STf")
                    nc.sync.dma_start(STf[:], cc_out[:])
                else:
                    STf = STf_p

                # ---- S, its A_re/A_im products, S@w0b ----
                S_ps = small_ps.tile([K, C], F32, tag="sps")
                nc.tensor.transpose(S_ps[:], STf[:], ident_f[:])
                S_bf = smalls.tile([K, C], BF, tag="Sbf")
                nc.scalar.activation(S_bf[:], S_ps[:], AF.Copy)
                Sre_ps = small_ps.tile([K, C], F32, tag="sps")
                nc.tensor.matmul(Sre_ps[:], STf[:], Are_s[i][:],
                                 start=True, stop=True)
                Sre_bf = smalls.tile([K, C], BF, tag="Srebf")
                nc.scalar.activation(Sre_bf[:], Sre_ps[:], AF.Copy)
                Sim_ps = small_ps.tile([K, C], F32, tag="sps")
                nc.tensor.matmul(Sim_ps[:], STf[:], Aim_s[i][:],
                                 start=True, stop=True)
                Sim_bf = smalls.tile([K, C], BF, tag="Simbf")
                nc.scalar.activation(Sim_bf[:], Sim_ps[:], AF.Copy)
                nSim_bf = smalls.tile([K, C], BF, tag="nSimbf")
                nc.vector.tensor_scalar_mul(nSim_bf[:], Sim_ps[:], -1.0)
                SW0b_ps = small_ps.tile([K, C], F32, tag="sps")
                nc.tensor.matmul(SW0b_ps[:], STf[:], w0bf_s[i][:],
                                 start=True, stop=True)
                SW0b_bf = smalls.tile([K, C], BF, tag="SW0b")
                nc.scalar.activation(SW0b_bf[:], SW0b_ps[:], AF.Copy)

                # ---- fused per-node sweep ----
                for cI in range(nch):
                    sl = bass.ts(cI, CH)
                    ev_c = stream.tile([K, CH], BF, tag="ev")
                    nc.sync.dma_start(ev_c[:], evT[:, sl])
                    gx_c = stream.tile([K, CH], BF, tag="gx")
                    nc.sync.dma_start(gx_c[:], gxT[:, sl])
                    gy_c = stream.tile([K, CH], BF, tag="gy")
                    nc.sync.dma_start(gy_c[:], gyT[:, sl])

                    gX_ps = mm_ps.tile([C, CH], F32, tag="mm")
                    nc.tensor.matmul(gX_ps[:], S_bf[:], gx_c[:],
                                     start=True, stop=True)
                    gY_ps = mm_ps.tile([C, CH], F32, tag="mm")
                    nc.tensor.matmul(gY_ps[:], S_bf[:], gy_c[:],
                                     start=True, stop=True)
                    Br_ps = mm_ps.tile([C, CH], F32, tag="mm")
                    nc.tensor.matmul(Br_ps[:], Sre_bf[:], gx_c[:],
                                     start=True, stop=False)
                    nc.tensor.matmul(Br_ps[:], nSim_bf[:], gy_c[:],
                                     start=False, stop=True)
                    Bi_ps = mm_ps.tile([C, CH], F32, tag="mm")
                    nc.tensor.matmul(Bi_ps[:], Sre_bf[:], gy_c[:],
                                     start=True, stop=False)
                    nc.tensor.matmul(Bi_ps[:], Sim_bf[:], gx_c[:],
                                     start=False, stop=True)

                    Br_sb = csb.tile([C, CH], BF, tag="Br")
                    nc.scalar.activation(Br_sb[:], Br_ps[:], AF.Copy)
                    Bi_sb = csb.tile([C, CH], BF, tag="Bi")
                    nc.vector.tensor_copy(Bi_sb[:], Bi_ps[:])
                    m1 = csb.tile([C, CH], BF, tag="m1")
                    nc.vector.tensor_mul(m1[:], gX_ps[:], Br_sb[:])
                    m2 = csb.tile([C, CH], BF, tag="m2")
                    nc.vector.tensor_mul(m2[:], gY_ps[:], Bi_sb[:])
                    a1 = csb.tile([C, CH], BF, tag="a1")
                    nc.vector.tensor_add(a1[:], m1[:], m2[:])
                    gf = csb.tile([C, CH], BF, tag="gf")
                    nc.scalar.activation(gf[:], a1[:], AF.Tanh)

                    h0_ps = mm_ps.tile([C, CH], F32, tag="mm")
                    nc.tensor.matmul(h0_ps[:], w0a_r[i][:], xs[cI][:],
                                     start=True, stop=False)
                    nc.tensor.matmul(h0_ps[:], SW0b_bf[:], ev_c[:],
                                     start=False, stop=False)
                    nc.tensor.matmul(h0_ps[:], w0c_s[i][:], gf[:],
                                     start=False, stop=True)
                    h0_sb = csb.tile([C, CH], BF, tag="h0")
                    nc.scalar.activation(h0_sb[:], h0_ps[:], AF.Relu,
                                         bias=b0_s[i][:])
                    h1_ps = mm_ps.tile([C, CH], F32, tag="mm")
                    nc.tensor.matmul(h1_ps[:], w1_s[i][:], h0_sb[:],
                                     start=True, stop=True)
                    h1_sb = csb.tile([C, CH], BF, tag="h1")
                    nc.scalar.activation(h1_sb[:], h1_ps[:], AF.Relu,
                                         bias=b1_s[i][:])
                    h2_ps = mm_ps.tile([C, CH], F32, tag="mm")
                    nc.tensor.matmul(h2_ps[:], w2_s[i][:], h1_sb[:],
                                     start=True, stop=True)
                    # x += h2 + b2 (x carried in fp32r)
                    nc.vector.scalar_tensor_tensor(
                        out=xs[cI][:], in0=h2_ps[:], scalar=b2_s[i][:],
                        in1=xs[cI][:], op0=ALU.add, op1=ALU.add)

            # ---- output head ----
            for cI in range(nch):
                sl = bass.ts(cI, CH)
                y_ps = mm_ps.tile([3, CH], F32, tag="mm")
                nc.tensor.matmul(y_ps[:], wlast_r[:], xs[cI][:],
                                 start=True, stop=True)
                y_sb = csb.tile([3, CH], F32, tag="y")
                nc.vector.tensor_scalar_add(y_sb[:], y_ps[:], blast_s[:])
                nc.sync.dma_start(yT[:, sl], y_sb[:])

    nc.compile()
    return nc


_NC_CACHE = {}


def _get_nc():
    if "nc" not in _NC_CACHE:
        _NC_CACHE["nc"] = build_nc()
    return _NC_CACHE["nc"]


def kernel(**inputs):
    nc = _get_nc()
    in_maps = host_prep(inputs)
    res = run_bass_kernel_spmd(nc, in_maps, core_ids=list(range(NCORES)))
    out = np.empty((B, N, 3), np.float32)
    for b in range(B):
        for h in range(2):
            yT = res.results[2 * b + h]["yT"]
            out[b, h * NH:(h + 1) * NH] = yT[:, :NH].T
    return out



# revision 30
# speedup vs baseline: 1.0481x; 1.0481x over previous
"""DiffusionNet forward on 8 Trainium2 NeuronCores.

Strategy (v3)
-------------
B=4 samples, 2 cores per sample, each core owns half the mesh nodes
(20000, zero-padded to 20480).  All cross-node coupling flows through the
K=128 spectral bottleneck:

  * SpMM eliminated on device: gX = (G @ evecs) @ S, host precomputes
    GXe = G @ evecs once per sample (exact associativity).
  * Everything big is SBUF-resident for the whole kernel (no per-block
    re-streaming): x (bf16), ev (bf16, K-major), evm (bf16, node-major)
    and the gradient operators (fp8e4, x64 scaled, [k, j, n] layout with
    j in {gx, gy}).
  * The four spectral-stream matmuls per chunk (gX, gY, Br, Bi) are each
    ONE fp8 DoubleRow matmul contracting 256 = K x {x,y}: the pair dim
    holds the gx/gy interleave, so Br = Sre@gx - Sim@gy needs no PSUM
    accumulation (accumulating DoubleRow pairs crash the runtime).
  * The forward spectral transform of the NEXT block is fused into the
    sweep: after the residual update of a chunk, its x tiles are
    transposed and immediately accumulated into the spectral partial
    (PSUM), split into two halves so the pair AllReduce of the first
    half hides under the second half of the sweep.
  * Block 1's spectrum is precomputed on host (full-sample sum), so no
    standalone forward pass and no AllReduce before the first sweep.
  * Elementwise work is spread over DVE (m1, m2, residual), Act (Br/Bi
    evictions, tanh, relus, casts) and Pool/gpsimd (a1 = m1 + m2).
"""

import sys
import numpy as np
import ml_dtypes

for _p in ("/opt/trn_rl_repo", "/root/.axon_site/_ro/trn_rl_repo"):
    if _p not in sys.path:
        sys.path.append(_p)

import concourse.bass as bass
import concourse.bacc as bacc
import concourse.tile as tile
import concourse.mybir as mybir
from concourse.bass_utils import run_bass_kernel_spmd
from concourse.masks import make_identity

BF = mybir.dt.bfloat16
F32 = mybir.dt.float32
F8 = mybir.dt.float8e4
AF = mybir.ActivationFunctionType
ALU = mybir.AluOpType
DR = mybir.MatmulPerfMode.DoubleRow

B, N, E, K = 4, 40000, 240000, 128
C = 128
NB = 4          # diffusion blocks
NCORES = 8
NH = N // 2     # nodes per core (half sample)
CH = 512        # node chunk (matmul free dim)
NHP = 20480     # padded nodes per core: 40 chunks * 512
NCH = NHP // CH
PAIRS = [[0, 1], [2, 3], [4, 5], [6, 7]]
SG = 64.0       # fp8 scale on GXe/GYe; tanh un-scales by 1/SG^2
BRIDGE_XBAR = False  # xbar-DMA transpose bridge vs PE transpose + DVE evict
POOL_A1 = True       # a1 on gpsimd/Pool vs DVE
USE_CC = True        # pairwise AllReduce vs local-only (debug)

bf16 = ml_dtypes.bfloat16
f8e4 = ml_dtypes.float8_e4m3


# ----------------------------------------------------------------- host side

def _spmm_mat(rows, cols, vals, M):
    """(COO [N,N] with given pattern) @ M, dense M [N,k]. Pure numpy."""
    out = np.zeros((N, M.shape[1]), np.float32)
    perm = np.argsort(rows, kind="stable")
    contrib = (vals[:, None] * M[cols]).astype(np.float32)[perm]
    rs = rows[perm]
    uniq, starts = np.unique(rs, return_index=True)
    out[uniq] = np.add.reduceat(contrib, starts, axis=0)
    return out


def host_prep(inputs, nhp=NHP, nb=NB):
    """Build the 8 per-core input dicts."""
    x_in = np.asarray(inputs["x_in"], np.float32)
    mass = np.asarray(inputs["mass"], np.float32)
    evals = np.asarray(inputs["evals"], np.float32)
    evecs = np.asarray(inputs["evecs"], np.float32)
    rows = np.asarray(inputs["rows"])
    cols = np.asarray(inputs["cols"])
    gX_vals = np.asarray(inputs["gradX_vals"], np.float32)
    gY_vals = np.asarray(inputs["gradY_vals"], np.float32)
    w_first = np.asarray(inputs["w_first"], np.float32)
    b_first = np.asarray(inputs["b_first"], np.float32)
    diff_time = np.asarray(inputs["diff_time"], np.float32)
    A_re = np.asarray(inputs["A_re"], np.float32)
    A_im = np.asarray(inputs["A_im"], np.float32)
    mlp_w0 = np.asarray(inputs["mlp_w0"], np.float32)
    w1 = np.asarray(inputs["mlp_w1"], np.float32)
    w2 = np.asarray(inputs["mlp_w2"], np.float32)
    b0 = np.asarray(inputs["mlp_b0"], np.float32)
    b1 = np.asarray(inputs["mlp_b1"], np.float32)
    b2 = np.asarray(inputs["mlp_b2"], np.float32)
    w_last = np.asarray(inputs["w_last"], np.float32)
    b_last = np.asarray(inputs["b_last"], np.float32)

    nh = NH

    shared = dict(
        Are=A_re[:nb].astype(bf16),
        Aim=A_im[:nb].astype(bf16),
        w0a=np.ascontiguousarray(mlp_w0[:nb, 0:C]).astype(bf16),
        w0b=np.ascontiguousarray(mlp_w0[:nb, C:2 * C]).astype(bf16),
        w0c=np.ascontiguousarray(mlp_w0[:nb, 2 * C:3 * C]).astype(bf16),
        w1=w1[:nb].astype(bf16),
        w2=w2[:nb].astype(bf16),
        b0=b0[:nb].reshape(nb, C, 1),
        b1=b1[:nb].reshape(nb, C, 1),
        b2=b2[:nb].reshape(nb, C, 1),
        wlast=w_last.astype(bf16),
        blast=b_last.reshape(3, 1),
    )

    in_maps = []
    for b in range(B):
        ev = evecs[b]
        evm_full = ev * mass[b][:, None]
        GXe = _spmm_mat(rows, cols, gX_vals[b], ev)
        GYe = _spmm_mat(rows, cols, gY_vals[b], ev)
        x0_full = x_in[b] @ w_first + b_first
        # spec for block 0, full-sample sum (both halves): [C, K]
        spec1 = (x0_full.T @ evm_full).astype(np.float32)
        # coefs[i][c,k] = exp(-evals[k] * diff_time[i][c])
        coefs = np.exp(-evals[b][None, None, :]
                       * diff_time[:nb, :, None]).astype(np.float32)
        for h in range(2):
            sl = slice(h * nh, (h + 1) * nh)

            def padT(M):  # [nh, D] -> [D, nhp]
                out = np.zeros((M.shape[1], nhp), np.float32)
                out[:, :nh] = M[sl].T
                return out

            # trio [nch, 128, 3, 512] bf16: 0 = x0, 1 = ev (K-major),
            # 2 = evm node-major tiles (t, k)
            x0c = padT(x0_full).reshape(C, NCH, CH).transpose(1, 0, 2)
            evc = padT(ev).reshape(K, NCH, CH).transpose(1, 0, 2)
            evmP = np.zeros((nhp, K), np.float32)
            evmP[:nh] = evm_full[sl]
            evmc = evmP.reshape(NCH, 4, 128, K).transpose(0, 2, 1, 3) \
                       .reshape(NCH, 128, CH)
            trio = np.stack([x0c, evc, evmc], axis=2).astype(bf16)

            # gxy [nch, 128(k), 2(j), 512(n)] fp8: j=0 gx, j=1 gy
            def kpack(Gm):  # [nh, K] -> [nch, 128, 512]
                GT = np.clip(padT(Gm) * SG, -240.0, 240.0)  # [K, nhp]
                return GT.reshape(K, NCH, CH).transpose(1, 0, 2)

            gxy = np.stack([kpack(GXe), kpack(GYe)], axis=2).astype(f8e4)

            in_maps.append(dict(
                trio=trio,
                gxy=np.ascontiguousarray(gxy),
                spec1=spec1,
                coefs=coefs,
                **shared,
            ))
    return in_maps


# --------------------------------------------------------------- device side

def build_nc(nb=NB, nch=NCH, ncores=NCORES):
    nhp = nch * CH
    nc = bacc.Bacc("TRN2", target_bir_lowering=False, debug=False,
                   enable_asserts=False, num_devices=ncores)

    trio = nc.dram_tensor("trio", [nch, 128, 3, CH], BF, kind="ExternalInput")
    gxy = nc.dram_tensor("gxy", [nch, 128, 2, CH], F8, kind="ExternalInput")
    spec1 = nc.dram_tensor("spec1", [C, K], F32, kind="ExternalInput")
    coefs = nc.dram_tensor("coefs", [nb, C, K], F32, kind="ExternalInput")
    Are = nc.dram_tensor("Are", [nb, C, C], BF, kind="ExternalInput")
    Aim = nc.dram_tensor("Aim", [nb, C, C], BF, kind="ExternalInput")
    w0a = nc.dram_tensor("w0a", [nb, C, C], BF, kind="ExternalInput")
    w0b = nc.dram_tensor("w0b", [nb, C, C], BF, kind="ExternalInput")
    w0c = nc.dram_tensor("w0c", [nb, C, C], BF, kind="ExternalInput")
    w1 = nc.dram_tensor("w1", [nb, C, C], BF, kind="ExternalInput")
    w2 = nc.dram_tensor("w2", [nb, C, C], BF, kind="ExternalInput")
    b0 = nc.dram_tensor("b0", [nb, C, 1], F32, kind="ExternalInput")
    b1 = nc.dram_tensor("b1", [nb, C, 1], F32, kind="ExternalInput")
    b2 = nc.dram_tensor("b2", [nb, C, 1], F32, kind="ExternalInput")
    wlast = nc.dram_tensor("wlast", [C, 3], BF, kind="ExternalInput")
    blast = nc.dram_tensor("blast", [3, 1], F32, kind="ExternalInput")
    yT = nc.dram_tensor("yT", [3, nhp], F32, kind="ExternalOutput")

    with tile.TileContext(nc) as tc:
        with (
            tc.tile_pool(name="consts", bufs=1) as consts,
            tc.tile_pool(name="res", bufs=1) as res,
            tc.tile_pool(name="wk", bufs=2) as wk,
            tc.tile_pool(name="sm", bufs=2) as sm,
            tc.tile_pool(name="pmm", bufs=6, space="PSUM") as pmm,
            tc.tile_pool(name="psm", bufs=1, space="PSUM") as psm,
            tc.tile_pool(name="dram", bufs=2, space="DRAM") as dram,
        ):
            ident_bf = consts.tile([128, 128], BF, tag="identb")
            make_identity(nc, ident_bf[:])

            def cload(src, shape, dt, tag):
                t = consts.tile(shape, dt, tag=tag)
                nc.sync.dma_start(t[:], src)
                return t

            coefs_s = [cload(coefs[i], [C, K], F32, f"cf{i}") for i in range(nb)]
            Are_s = [cload(Are[i], [C, C], BF, f"Are{i}") for i in range(nb)]
            Aim_s = [cload(Aim[i], [C, C], BF, f"Aim{i}") for i in range(nb)]
            w0a_s = [cload(w0a[i], [C, C], BF, f"w0a{i}") for i in range(nb)]
            w0b_s = [cload(w0b[i], [C, C], BF, f"w0b{i}") for i in range(nb)]
            w0c_s = [cload(w0c[i], [C, C], BF, f"w0c{i}") for i in range(nb)]
            w1_s = [cload(w1[i], [C, C], BF, f"w1{i}") for i in range(nb)]
            w2_s = [cload(w2[i], [C, C], BF, f"w2{i}") for i in range(nb)]
            b0_s = [cload(b0[i], [C, 1], F32, f"b0{i}") for i in range(nb)]
            b1_s = [cload(b1[i], [C, 1], F32, f"b1{i}") for i in range(nb)]
            b2_s = [cload(b2[i], [C, 1], F32, f"b2{i}") for i in range(nb)]
            wlast_s = cload(wlast[:], [C, 3], BF, "wlast")
            blast_s = cload(blast[:], [3, 1], F32, "blast")
            spec1_s = cload(spec1[:], [C, K], F32, "spec1")

            # resident per-chunk tiles, streamed in once
            trio_t, gxy_t = [], []
            for cI in range(nch):
                t = res.tile([128, 3, CH], BF, tag=f"trio{cI}")
                nc.sync.dma_start(t[:], trio[cI])
                trio_t.append(t)
                g = res.tile([128, 2, CH], F8, tag=f"gxy{cI}")
                nc.sync.dma_start(g[:], gxy[cI])
                gxy_t.append(g)

            # PSUM scratch banks for small accumulators, manually packed
            s1 = psm.tile([128, CH], F32, tag="s1")
            s2 = psm.tile([128, CH], F32, tag="s2")
            specA_ps = s1[:, 0:128]
            specB_ps = s1[:, 128:256]
            sw0b_ps = s1[:, 256:384]                      # [K, C] f32
            sretT_ps = s2[:, 0:128]                       # Sre^T [C, K] f32
            simtT_ps = s2[:, 128:256]                     # Sim^T [C, K] f32
            bfh = s2[:, 256:384].bitcast(BF)              # [128, 256] bf16
            S_ps = bfh[:, 0:128]                          # S [K, C] bf16
            Sre_ps = bfh[:, 128:256]                      # Sre [K, C] bf16
            Sim_ps = s2[:, 384:448].bitcast(BF)           # Sim [K, C] bf16

            if USE_CC:
                # collective warmup (ring spin-up off the critical path)
                warm_sb = sm.tile([C, K], F32, tag="warm", bufs=1)
                nc.gpsimd.memset(warm_sb[:], 0.0)
                cc_wi = dram.tile([C, K], F32, tag="cwi", bufs=1)
                cc_wo = dram.tile([C, K], F32, tag="cwo", bufs=1)
                nc.sync.dma_start(cc_wi[:], warm_sb[:])
                nc.gpsimd.collective_compute(
                    "AllReduce", ALU.add, replica_groups=PAIRS[:ncores // 2],
                    ins=[cc_wi.opt()], outs=[cc_wo.opt()])

            for i in range(nb):
                # ---- smalls: DoubleRow W matrices from the block spectrum
                #      WgX=[S|0] WgY=[0|S] WBr=[Sre|-Sim] WBi=[Sim|Sre],
                #      each [k, j, c] fp8 with j the gx/gy pair dim ----
                if i == 0:
                    spec_f = spec1_s
                else:
                    sA = sm.tile([C, K], F32, tag="sA")
                    nc.sync.dma_start(sA[:], cc_oA[:])
                    sB = sm.tile([C, K], F32, tag="sB")
                    nc.sync.dma_start(sB[:], cc_oB[:])
                    spec_f = sm.tile([C, K], F32, tag="spec")
                    nc.vector.tensor_add(spec_f[:], sA[:], sB[:])

                stf_bf = sm.tile([C, K], BF, tag="stfb")
                nc.vector.tensor_mul(stf_bf[:], spec_f[:], coefs_s[i][:])
                # S = (S^T)^T first: WgX/WgY unblock the sweep
                nc.tensor.matmul(S_ps, stf_bf[:], ident_bf[:],
                                 is_transpose=True, skip_group_check=True)
                wgx = sm.tile([128, 2, 128], F8, tag="wgx")
                nc.scalar.activation(wgx[:, 0, :], S_ps, AF.Copy)
                nc.scalar.activation(wgx[:, 1, :], S_ps, AF.Copy, scale=0.0)
                wgy = sm.tile([128, 2, 128], F8, tag="wgy")
                nc.scalar.activation(wgy[:, 0, :], S_ps, AF.Copy, scale=0.0)
                nc.scalar.activation(wgy[:, 1, :], S_ps, AF.Copy)

                nc.tensor.matmul(sretT_ps, Are_s[i][:], stf_bf[:],
                                 start=True, stop=True, skip_group_check=True)
                nc.tensor.matmul(simtT_ps, Aim_s[i][:], stf_bf[:],
                                 start=True, stop=True, skip_group_check=True)
                sret_bf = sm.tile([C, K], BF, tag="sreb")
                nc.scalar.activation(sret_bf[:], sretT_ps, AF.Copy)
                simt_bf = sm.tile([C, K], BF, tag="simb")
                nc.scalar.activation(simt_bf[:], simtT_ps, AF.Copy)
                nc.tensor.matmul(Sre_ps, sret_bf[:], ident_bf[:],
                                 is_transpose=True, skip_group_check=True)
                nc.tensor.matmul(Sim_ps, simt_bf[:], ident_bf[:],
                                 is_transpose=True, skip_group_check=True)
                wbr = sm.tile([128, 2, 128], F8, tag="wbr")
                nc.scalar.activation(wbr[:, 0, :], Sre_ps, AF.Copy)
                nc.scalar.activation(wbr[:, 1, :], Sim_ps, AF.Copy, scale=-1.0)
                wbi = sm.tile([128, 2, 128], F8, tag="wbi")
                nc.scalar.activation(wbi[:, 0, :], Sim_ps, AF.Copy)
                nc.scalar.activation(wbi[:, 1, :], Sre_ps, AF.Copy)

                nc.tensor.matmul(sw0b_ps, stf_bf[:], w0b_s[i][:],
                                 start=True, stop=True, skip_group_check=True)
                sw0b_bf = sm.tile([K, C], BF, tag="sw0b")
                nc.scalar.activation(sw0b_bf[:], sw0b_ps, AF.Copy)

                if i < nb - 1:
                    cc_iA = dram.tile([C, K], F32, tag="ciA")
                    cc_oA = dram.tile([C, K], F32, tag="coA")
                    cc_iB = dram.tile([C, K], F32, tag="ciB")
                    cc_oB = dram.tile([C, K], F32, tag="coB")

                # ---- fused sweep over node chunks ----
                for cI in range(nch):
                    x_ap = trio_t[cI][:, 0, :]
                    ev_ap = trio_t[cI][:, 1, :]
                    g_ap = gxy_t[cI][:]

                    gX = pmm.tile([C, CH], F32, tag="mm")
                    nc.tensor.matmul(gX[:], wgx[:], g_ap,
                                     start=True, stop=True, perf_mode=DR)
                    gY = pmm.tile([C, CH], F32, tag="mm")
                    nc.tensor.matmul(gY[:], wgy[:], g_ap,
                                     start=True, stop=True, perf_mode=DR)
                    Br = pmm.tile([C, CH], F32, tag="mm")
                    nc.tensor.matmul(Br[:], wbr[:], g_ap,
                                     start=True, stop=True, perf_mode=DR)
                    Bi = pmm.tile([C, CH], F32, tag="mm")
                    nc.tensor.matmul(Bi[:], wbi[:], g_ap,
                                     start=True, stop=True, perf_mode=DR)

                    br_sb = wk.tile([C, CH], BF, tag="brs")
                    nc.scalar.activation(br_sb[:], Br[:], AF.Copy)
                    bi_sb = wk.tile([C, CH], BF, tag="bis")
                    nc.scalar.activation(bi_sb[:], Bi[:], AF.Copy)
                    m1 = wk.tile([C, CH], BF, tag="m1")
                    nc.vector.tensor_mul(m1[:], gX[:], br_sb[:])
                    m2 = wk.tile([C, CH], BF, tag="m2")
                    nc.vector.tensor_mul(m2[:], gY[:], bi_sb[:])
                    a1 = wk.tile([C, CH], BF, tag="a1")
                    if POOL_A1:
                        nc.gpsimd.tensor_add(a1[:], m1[:], m2[:])
                    else:
                        nc.vector.tensor_add(a1[:], m1[:], m2[:])
                    gf = wk.tile([C, CH], BF, tag="gf")
                    nc.scalar.activation(gf[:], a1[:], AF.Tanh,
                                         scale=1.0 / (SG * SG))

                    h0 = pmm.tile([C, CH], F32, tag="mm")
                    nc.tensor.matmul(h0[:], w0a_s[i][:], x_ap,
                                     start=True, stop=False)
                    nc.tensor.matmul(h0[:], sw0b_bf[:], ev_ap,
                                     start=False, stop=False)
                    nc.tensor.matmul(h0[:], w0c_s[i][:], gf[:],
                                     start=False, stop=True)
                    h0s = wk.tile([C, CH], BF, tag="h0s")
                    nc.scalar.activation(h0s[:], h0[:], AF.Relu,
                                         bias=b0_s[i][:])
                    h1 = pmm.tile([C, CH], F32, tag="mm")
                    nc.tensor.matmul(h1[:], w1_s[i][:], h0s[:],
                                     start=True, stop=True)
                    h1s = wk.tile([C, CH], BF, tag="h1s")
                    nc.scalar.activation(h1s[:], h1[:], AF.Relu,
                                         bias=b1_s[i][:])
                    h2 = pmm.tile([C, CH], F32, tag="mm")
                    nc.tensor.matmul(h2[:], w2_s[i][:], h1s[:],
                                     start=True, stop=True)
                    # x += h2 + b2 (bf16 residual carrier)
                    nc.vector.scalar_tensor_tensor(
                        out=x_ap, in0=h2[:], scalar=b2_s[i][:],
                        in1=x_ap, op0=ALU.add, op1=ALU.add)

                    if i < nb - 1:
                        # bridge: transpose x_new into 4 stacked 128x128
                        # node-major tiles, accumulate spectral delta on PE
                        xnm = wk.tile([128, 4, 128], BF, tag="xnm")
                        if BRIDGE_XBAR:
                            nc.sync.dma_start_transpose(xnm[:], x_ap)
                        else:
                            hT = pmm.tile([128, 4, 128], BF, tag="mm")
                            for t in range(4):
                                nc.tensor.transpose(
                                    hT[:, t, :],
                                    x_ap[:, t * 128:(t + 1) * 128],
                                    ident_bf[:])
                            nc.vector.tensor_copy(xnm[:], hT[:])
                        spec_dst = specA_ps if cI < nch // 2 else specB_ps
                        first = cI in (0, nch // 2)
                        last = cI in (nch // 2 - 1, nch - 1)
                        for t in range(4):
                            nc.tensor.matmul(
                                spec_dst,
                                xnm[:, t, :],
                                trio_t[cI][:, 2, t * 128:(t + 1) * 128],
                                start=(first and t == 0),
                                stop=(last and t == 3),
                                skip_group_check=True)
                        if cI == nch // 2 - 1:
                            spA = sm.tile([C, K], F32, tag="spA")
                            nc.vector.tensor_copy(spA[:], specA_ps)
                            nc.sync.dma_start(cc_iA[:], spA[:])
                            if USE_CC:
                                nc.gpsimd.collective_compute(
                                    "AllReduce", ALU.add,
                                    replica_groups=PAIRS[:ncores // 2],
                                    ins=[cc_iA.opt()], outs=[cc_oA.opt()])
                            else:
                                nc.sync.dma_start(cc_oA[:], spA[:])
                        elif cI == nch - 1:
                            spB = sm.tile([C, K], F32, tag="spB")
                            nc.vector.tensor_copy(spB[:], specB_ps)
                            nc.sync.dma_start(cc_iB[:], spB[:])
                            if USE_CC:
                                nc.gpsimd.collective_compute(
                                    "AllReduce", ALU.add,
                                    replica_groups=PAIRS[:ncores // 2],
                                    ins=[cc_iB.opt()], outs=[cc_oB.opt()])
                            else:
                                nc.sync.dma_start(cc_oB[:], spB[:])
                    else:
                        # output head
                        y = pmm.tile([3, CH], F32, tag="mm")
                        nc.tensor.matmul(y[:], wlast_s[:], x_ap,
                                         start=True, stop=True)
                        ysb = wk.tile([3, CH], F32, tag="y")
                        nc.vector.tensor_scalar_add(ysb[:], y[:], blast_s[:])
                        nc.sync.dma_start(yT[:, cI * CH:(cI + 1) * CH], ysb[:])

    nc.compile()
    return nc


_NC_CACHE = {}


def _get_nc():
    if "nc" not in _NC_CACHE:
        _NC_CACHE["nc"] = build_nc()
    return _NC_CACHE["nc"]


def kernel(**inputs):
    nc = _get_nc()
    in_maps = host_prep(inputs)
    res = run_bass_kernel_spmd(nc, in_maps, core_ids=list(range(NCORES)))
    out = np.empty((B, N, 3), np.float32)
    for b in range(B):
        for h in range(2):
            yT = res.results[2 * b + h]["yT"]
            out[b, h * NH:(h + 1) * NH] = yT[:, :NH].T
    return out


# revision 31
# speedup vs baseline: 1.4956x; 1.4269x over previous
"""DiffusionNet forward on 8 Trainium2 NeuronCores.

Strategy (v3)
-------------
B=4 samples, 2 cores per sample, each core owns half the mesh nodes
(20000, zero-padded to 20480).  All cross-node coupling flows through the
K=128 spectral bottleneck:

  * SpMM eliminated on device: gX = (G @ evecs) @ S, host precomputes
    GXe = G @ evecs once per sample (exact associativity).
  * Everything big is SBUF-resident for the whole kernel (no per-block
    re-streaming): x (bf16), ev (bf16, K-major), evm (bf16, node-major)
    and the gradient operators (fp8e4, x64 scaled, [k, j, n] layout with
    j in {gx, gy}).
  * The four spectral-stream matmuls per chunk (gX, gY, Br, Bi) are each
    ONE fp8 DoubleRow matmul contracting 256 = K x {x,y}: the pair dim
    holds the gx/gy interleave, so Br = Sre@gx - Sim@gy needs no PSUM
    accumulation (accumulating DoubleRow pairs crash the runtime).
  * The forward spectral transform of the NEXT block is fused into the
    sweep: after the residual update of a chunk, its x tiles are
    transposed and immediately accumulated into the spectral partial
    (PSUM), split into two halves so the pair AllReduce of the first
    half hides under the second half of the sweep.
  * Block 1's spectrum is precomputed on host (full-sample sum), so no
    standalone forward pass and no AllReduce before the first sweep.
  * Elementwise work is spread over DVE (m1, m2, residual), Act (Br/Bi
    evictions, tanh, relus, casts) and Pool/gpsimd (a1 = m1 + m2).
"""

import sys
import numpy as np
import ml_dtypes

for _p in ("/opt/trn_rl_repo", "/root/.axon_site/_ro/trn_rl_repo"):
    if _p not in sys.path:
        sys.path.append(_p)

import concourse.bass as bass
import concourse.bacc as bacc
import concourse.tile as tile
import concourse.mybir as mybir
from concourse.bass_utils import run_bass_kernel_spmd
from concourse.masks import make_identity

BF = mybir.dt.bfloat16
F32 = mybir.dt.float32
F8 = mybir.dt.float8e4
AF = mybir.ActivationFunctionType
ALU = mybir.AluOpType
DR = mybir.MatmulPerfMode.DoubleRow

B, N, E, K = 4, 40000, 240000, 128
C = 128
NB = 4          # diffusion blocks
NCORES = 8
NH = N // 2     # nodes per core (half sample)
CH = 512        # node chunk (matmul free dim)
NHP = 20480     # padded nodes per core: 40 chunks * 512
NCH = NHP // CH
PAIRS = [[0, 1], [2, 3], [4, 5], [6, 7]]
SG = 64.0       # fp8 scale on GXe/GYe; tanh un-scales by 1/SG^2
BRIDGE_XBAR = False  # xbar-DMA transpose bridge vs PE transpose + DVE evict
POOL_A1 = True       # a1 on gpsimd/Pool vs DVE
USE_CC = True        # pairwise AllReduce vs local-only (debug)

bf16 = ml_dtypes.bfloat16
f8e4 = ml_dtypes.float8_e4m3


# ----------------------------------------------------------------- host side

def _spmm_mat(rows, cols, vals, M):
    """(COO [N,N] with given pattern) @ M, dense M [N,k]. Pure numpy."""
    out = np.zeros((N, M.shape[1]), np.float32)
    perm = np.argsort(rows, kind="stable")
    contrib = (vals[:, None] * M[cols]).astype(np.float32)[perm]
    rs = rows[perm]
    uniq, starts = np.unique(rs, return_index=True)
    out[uniq] = np.add.reduceat(contrib, starts, axis=0)
    return out


def host_prep(inputs, nhp=NHP, nb=NB):
    """Build the 8 per-core input dicts."""
    x_in = np.asarray(inputs["x_in"], np.float32)
    mass = np.asarray(inputs["mass"], np.float32)
    evals = np.asarray(inputs["evals"], np.float32)
    evecs = np.asarray(inputs["evecs"], np.float32)
    rows = np.asarray(inputs["rows"])
    cols = np.asarray(inputs["cols"])
    gX_vals = np.asarray(inputs["gradX_vals"], np.float32)
    gY_vals = np.asarray(inputs["gradY_vals"], np.float32)
    w_first = np.asarray(inputs["w_first"], np.float32)
    b_first = np.asarray(inputs["b_first"], np.float32)
    diff_time = np.asarray(inputs["diff_time"], np.float32)
    A_re = np.asarray(inputs["A_re"], np.float32)
    A_im = np.asarray(inputs["A_im"], np.float32)
    mlp_w0 = np.asarray(inputs["mlp_w0"], np.float32)
    w1 = np.asarray(inputs["mlp_w1"], np.float32)
    w2 = np.asarray(inputs["mlp_w2"], np.float32)
    b0 = np.asarray(inputs["mlp_b0"], np.float32)
    b1 = np.asarray(inputs["mlp_b1"], np.float32)
    b2 = np.asarray(inputs["mlp_b2"], np.float32)
    w_last = np.asarray(inputs["w_last"], np.float32)
    b_last = np.asarray(inputs["b_last"], np.float32)

    nh = NH

    shared = dict(
        Are=A_re[:nb].astype(bf16),
        Aim=A_im[:nb].astype(bf16),
        w0a=np.ascontiguousarray(mlp_w0[:nb, 0:C]).astype(bf16),
        w0b=np.ascontiguousarray(mlp_w0[:nb, C:2 * C]).astype(bf16),
        w0c=np.ascontiguousarray(mlp_w0[:nb, 2 * C:3 * C]).astype(bf16),
        w1=w1[:nb].astype(bf16),
        w2=w2[:nb].astype(bf16),
        b0=b0[:nb].reshape(nb, C, 1),
        b1=b1[:nb].reshape(nb, C, 1),
        b2=b2[:nb].reshape(nb, C, 1),
        wlast=w_last.astype(bf16),
        blast=b_last.reshape(3, 1),
    )

    in_maps = []
    for b in range(B):
        ev = evecs[b]
        evm_full = ev * mass[b][:, None]
        GXe = _spmm_mat(rows, cols, gX_vals[b], ev)
        GYe = _spmm_mat(rows, cols, gY_vals[b], ev)
        x0_full = x_in[b] @ w_first + b_first
        # spec for block 0, full-sample sum (both halves): [C, K]
        spec1 = (x0_full.T @ evm_full).astype(np.float32)
        # coefs[i][c,k] = exp(-evals[k] * diff_time[i][c])
        coefs = np.exp(-evals[b][None, None, :]
                       * diff_time[:nb, :, None]).astype(np.float32)
        for h in range(2):
            sl = slice(h * nh, (h + 1) * nh)

            def padT(M):  # [nh, D] -> [D, nhp]
                out = np.zeros((M.shape[1], nhp), np.float32)
                out[:, :nh] = M[sl].T
                return out

            # trio [nch, 128, 3, 512] bf16: 0 = x0, 1 = ev (K-major),
            # 2 = evm node-major tiles (t, k)
            x0c = padT(x0_full).reshape(C, NCH, CH).transpose(1, 0, 2)
            evc = padT(ev).reshape(K, NCH, CH).transpose(1, 0, 2)
            evmP = np.zeros((nhp, K), np.float32)
            evmP[:nh] = evm_full[sl]
            evmc = evmP.reshape(NCH, 4, 128, K).transpose(0, 2, 1, 3) \
                       .reshape(NCH, 128, CH)
            trio = np.stack([x0c, evc, evmc], axis=2).astype(bf16)

            # gxy [nch, 128(k), 2(j), 512(n)] fp8: j=0 gx, j=1 gy
            def kpack(Gm):  # [nh, K] -> [nch, 128, 512]
                GT = np.clip(padT(Gm) * SG, -240.0, 240.0)  # [K, nhp]
                return GT.reshape(K, NCH, CH).transpose(1, 0, 2)

            gxy = np.stack([kpack(GXe), kpack(GYe)], axis=2).astype(f8e4)

            in_maps.append(dict(
                trio=trio,
                gxy=np.ascontiguousarray(gxy),
                spec1=spec1,
                coefs=coefs,
                **shared,
            ))
    return in_maps


# --------------------------------------------------------------- device side

def build_nc(nb=NB, nch=NCH, ncores=NCORES):
    nhp = nch * CH
    nc = bacc.Bacc("TRN2", target_bir_lowering=False, debug=False,
                   enable_asserts=False, num_devices=ncores)

    trio = nc.dram_tensor("trio", [nch, 128, 3, CH], BF, kind="ExternalInput")
    gxy = nc.dram_tensor("gxy", [nch, 128, 2, CH], F8, kind="ExternalInput")
    spec1 = nc.dram_tensor("spec1", [C, K], F32, kind="ExternalInput")
    coefs = nc.dram_tensor("coefs", [nb, C, K], F32, kind="ExternalInput")
    Are = nc.dram_tensor("Are", [nb, C, C], BF, kind="ExternalInput")
    Aim = nc.dram_tensor("Aim", [nb, C, C], BF, kind="ExternalInput")
    w0a = nc.dram_tensor("w0a", [nb, C, C], BF, kind="ExternalInput")
    w0b = nc.dram_tensor("w0b", [nb, C, C], BF, kind="ExternalInput")
    w0c = nc.dram_tensor("w0c", [nb, C, C], BF, kind="ExternalInput")
    w1 = nc.dram_tensor("w1", [nb, C, C], BF, kind="ExternalInput")
    w2 = nc.dram_tensor("w2", [nb, C, C], BF, kind="ExternalInput")
    b0 = nc.dram_tensor("b0", [nb, C, 1], F32, kind="ExternalInput")
    b1 = nc.dram_tensor("b1", [nb, C, 1], F32, kind="ExternalInput")
    b2 = nc.dram_tensor("b2", [nb, C, 1], F32, kind="ExternalInput")
    wlast = nc.dram_tensor("wlast", [C, 3], BF, kind="ExternalInput")
    blast = nc.dram_tensor("blast", [3, 1], F32, kind="ExternalInput")
    yT = nc.dram_tensor("yT", [3, nhp], F32, kind="ExternalOutput")

    with tile.TileContext(nc) as tc:
        with (
            tc.tile_pool(name="consts", bufs=1) as consts,
            tc.tile_pool(name="res", bufs=1) as res,
            tc.tile_pool(name="wk", bufs=2) as wk,
            tc.tile_pool(name="sm", bufs=2) as sm,
            tc.tile_pool(name="pmm", bufs=6, space="PSUM") as pmm,
            tc.tile_pool(name="psm", bufs=1, space="PSUM") as psm,
            tc.tile_pool(name="dram", bufs=2, space="DRAM") as dram,
        ):
            ident_bf = consts.tile([128, 128], BF, tag="identb")
            make_identity(nc, ident_bf[:])

            def cload(src, shape, dt, tag):
                t = consts.tile(shape, dt, tag=tag)
                nc.sync.dma_start(t[:], src)
                return t

            coefs_s = [cload(coefs[i], [C, K], F32, f"cf{i}") for i in range(nb)]
            Are_s = [cload(Are[i], [C, C], BF, f"Are{i}") for i in range(nb)]
            Aim_s = [cload(Aim[i], [C, C], BF, f"Aim{i}") for i in range(nb)]
            w0a_s = [cload(w0a[i], [C, C], BF, f"w0a{i}") for i in range(nb)]
            w0b_s = [cload(w0b[i], [C, C], BF, f"w0b{i}") for i in range(nb)]
            w0c_s = [cload(w0c[i], [C, C], BF, f"w0c{i}") for i in range(nb)]
            w1_s = [cload(w1[i], [C, C], BF, f"w1{i}") for i in range(nb)]
            w2_s = [cload(w2[i], [C, C], BF, f"w2{i}") for i in range(nb)]
            b0_s = [cload(b0[i], [C, 1], F32, f"b0{i}") for i in range(nb)]
            b1_s = [cload(b1[i], [C, 1], F32, f"b1{i}") for i in range(nb)]
            b2_s = [cload(b2[i], [C, 1], F32, f"b2{i}") for i in range(nb)]
            wlast_s = cload(wlast[:], [C, 3], BF, "wlast")
            blast_s = cload(blast[:], [3, 1], F32, "blast")
            spec1_s = cload(spec1[:], [C, K], F32, "spec1")

            # resident per-chunk tiles, streamed in once
            trio_t, gxy_t = [], []
            for cI in range(nch):
                t = res.tile([128, 3, CH], BF, tag=f"trio{cI}")
                nc.sync.dma_start(t[:], trio[cI])
                trio_t.append(t)
                g = res.tile([128, 2, CH], F8, tag=f"gxy{cI}")
                nc.sync.dma_start(g[:], gxy[cI])
                gxy_t.append(g)

            # PSUM scratch banks for small accumulators, manually packed
            s1 = psm.tile([128, CH], F32, tag="s1")
            s2 = psm.tile([128, CH], F32, tag="s2")
            specA_ps = s1[:, 0:128]
            specB_ps = s1[:, 128:256]
            sw0b_ps = s1[:, 256:384]                      # [K, C] f32
            sretT_ps = s2[:, 0:128]                       # Sre^T [C, K] f32
            simtT_ps = s2[:, 128:256]                     # Sim^T [C, K] f32
            bfh = s2[:, 256:384].bitcast(BF)              # [128, 256] bf16
            S_ps = bfh[:, 0:128]                          # S [K, C] bf16
            Sre_ps = bfh[:, 128:256]                      # Sre [K, C] bf16
            Sim_ps = s2[:, 384:448].bitcast(BF)           # Sim [K, C] bf16

            if USE_CC:
                # collective warmup (ring spin-up off the critical path)
                warm_sb = sm.tile([C, K], F32, tag="warm", bufs=1)
                nc.gpsimd.memset(warm_sb[:], 0.0)
                cc_wi = dram.tile([C, K], F32, tag="cwi", bufs=1)
                cc_wo = dram.tile([C, K], F32, tag="cwo", bufs=1)
                nc.sync.dma_start(cc_wi[:], warm_sb[:])
                nc.gpsimd.collective_compute(
                    "AllReduce", ALU.add, replica_groups=PAIRS[:ncores // 2],
                    ins=[cc_wi.opt()], outs=[cc_wo.opt()])

            for i in range(nb):
                # ---- smalls: DoubleRow W matrices from the block spectrum
                #      WgX=[S|0] WgY=[0|S] WBr=[Sre|-Sim] WBi=[Sim|Sre],
                #      each [k, j, c] fp8 with j the gx/gy pair dim ----
                if i == 0:
                    spec_f = spec1_s
                else:
                    sA = sm.tile([C, K], F32, tag="sA")
                    nc.sync.dma_start(sA[:], cc_oA[:])
                    sB = sm.tile([C, K], F32, tag="sB")
                    nc.sync.dma_start(sB[:], cc_oB[:])
                    spec_f = sm.tile([C, K], F32, tag="spec")
                    nc.vector.tensor_add(spec_f[:], sA[:], sB[:])

                stf_bf = sm.tile([C, K], BF, tag="stfb")
                nc.vector.tensor_mul(stf_bf[:], spec_f[:], coefs_s[i][:])
                # S = (S^T)^T first: WgX/WgY unblock the sweep
                nc.tensor.matmul(S_ps, stf_bf[:], ident_bf[:],
                                 is_transpose=True, skip_group_check=True)
                wgx = sm.tile([128, 2, 128], F8, tag="wgx")
                nc.scalar.activation(wgx[:, 0, :], S_ps, AF.Copy)
                nc.scalar.activation(wgx[:, 1, :], S_ps, AF.Copy, scale=0.0)
                wgy = sm.tile([128, 2, 128], F8, tag="wgy")
                nc.scalar.activation(wgy[:, 0, :], S_ps, AF.Copy, scale=0.0)
                nc.scalar.activation(wgy[:, 1, :], S_ps, AF.Copy)

                nc.tensor.matmul(sretT_ps, Are_s[i][:], stf_bf[:],
                                 start=True, stop=True, skip_group_check=True)
                nc.tensor.matmul(simtT_ps, Aim_s[i][:], stf_bf[:],
                                 start=True, stop=True, skip_group_check=True)
                sret_bf = sm.tile([C, K], BF, tag="sreb")
                nc.scalar.activation(sret_bf[:], sretT_ps, AF.Copy)
                simt_bf = sm.tile([C, K], BF, tag="simb")
                nc.scalar.activation(simt_bf[:], simtT_ps, AF.Copy)
                nc.tensor.matmul(Sre_ps, sret_bf[:], ident_bf[:],
                                 is_transpose=True, skip_group_check=True)
                nc.tensor.matmul(Sim_ps, simt_bf[:], ident_bf[:],
                                 is_transpose=True, skip_group_check=True)
                wbr = sm.tile([128, 2, 128], F8, tag="wbr")
                nc.scalar.activation(wbr[:, 0, :], Sre_ps, AF.Copy)
                nc.scalar.activation(wbr[:, 1, :], Sim_ps, AF.Copy, scale=-1.0)
                wbi = sm.tile([128, 2, 128], F8, tag="wbi")
                nc.scalar.activation(wbi[:, 0, :], Sim_ps, AF.Copy)
                nc.scalar.activation(wbi[:, 1, :], Sre_ps, AF.Copy)

                nc.tensor.matmul(sw0b_ps, stf_bf[:], w0b_s[i][:],
                                 start=True, stop=True, skip_group_check=True)
                sw0b_bf = sm.tile([K, C], BF, tag="sw0b")
                nc.scalar.activation(sw0b_bf[:], sw0b_ps, AF.Copy)

                if i < nb - 1:
                    cc_iA = dram.tile([C, K], F32, tag="ciA")
                    cc_oA = dram.tile([C, K], F32, tag="coA")
                    cc_iB = dram.tile([C, K], F32, tag="ciB")
                    cc_oB = dram.tile([C, K], F32, tag="coB")

                # ---- fused sweep over node chunks, software-pipelined:
                #      stage A(k) = quad + gX/gY evictions + m1/m2/a1,
                #      stage B(k) = tanh + MLP + relus + residual + bridge.
                #      Emitting A(k+1) before B(k) keeps the PE queue from
                #      blocking on the gf dependency. ----
                def stage_a(cI):
                    g_ap = gxy_t[cI][:]
                    gX = pmm.tile([C, CH], F32, tag="mm", name=f"gX{cI}")
                    nc.tensor.matmul(gX[:], wgx[:], g_ap,
                                     start=True, stop=True, perf_mode=DR)
                    gY = pmm.tile([C, CH], F32, tag="mm", name=f"gY{cI}")
                    nc.tensor.matmul(gY[:], wgy[:], g_ap,
                                     start=True, stop=True, perf_mode=DR)
                    gx_sb = wk.tile([C, CH], BF, tag="gxs", name=f"gxs{cI}")
                    nc.scalar.activation(gx_sb[:], gX[:], AF.Copy)
                    gy_sb = wk.tile([C, CH], BF, tag="gys", name=f"gys{cI}")
                    nc.scalar.activation(gy_sb[:], gY[:], AF.Copy)
                    Br = pmm.tile([C, CH], F32, tag="mm", name=f"Br{cI}")
                    nc.tensor.matmul(Br[:], wbr[:], g_ap,
                                     start=True, stop=True, perf_mode=DR)
                    Bi = pmm.tile([C, CH], F32, tag="mm", name=f"Bi{cI}")
                    nc.tensor.matmul(Bi[:], wbi[:], g_ap,
                                     start=True, stop=True, perf_mode=DR)
                    m1 = wk.tile([C, CH], BF, tag="m1", name=f"m1_{cI}")
                    nc.vector.tensor_mul(m1[:], Br[:], gx_sb[:])
                    m2 = wk.tile([C, CH], BF, tag="m2", name=f"m2_{cI}")
                    nc.vector.tensor_mul(m2[:], Bi[:], gy_sb[:])
                    a1 = wk.tile([C, CH], BF, tag="a1", name=f"a1_{cI}")
                    if POOL_A1:
                        nc.gpsimd.tensor_add(a1[:], m1[:], m2[:])
                    else:
                        nc.vector.tensor_add(a1[:], m1[:], m2[:])
                    return a1

                def stage_b(cI, a1):
                    x_ap = trio_t[cI][:, 0, :]
                    ev_ap = trio_t[cI][:, 1, :]
                    gf = wk.tile([C, CH], BF, tag="gf", name=f"gf{cI}")
                    nc.scalar.activation(gf[:], a1[:], AF.Tanh,
                                         scale=1.0 / (SG * SG))
                    h0 = pmm.tile([C, CH], F32, tag="mm", name=f"h0_{cI}")
                    nc.tensor.matmul(h0[:], w0a_s[i][:], x_ap,
                                     start=True, stop=False)
                    nc.tensor.matmul(h0[:], sw0b_bf[:], ev_ap,
                                     start=False, stop=False)
                    nc.tensor.matmul(h0[:], w0c_s[i][:], gf[:],
                                     start=False, stop=True)
                    h0s = wk.tile([C, CH], BF, tag="h0s", name=f"h0s{cI}")
                    nc.scalar.activation(h0s[:], h0[:], AF.Relu,
                                         bias=b0_s[i][:])
                    h1 = pmm.tile([C, CH], F32, tag="mm", name=f"h1_{cI}")
                    nc.tensor.matmul(h1[:], w1_s[i][:], h0s[:],
                                     start=True, stop=True)
                    h1s = wk.tile([C, CH], BF, tag="h1s", name=f"h1s{cI}")
                    nc.vector.tensor_scalar(h1s[:], h1[:], b1_s[i][:], 0.0,
                                            ALU.add, ALU.max)
                    h2 = pmm.tile([C, CH], F32, tag="mm", name=f"h2_{cI}")
                    nc.tensor.matmul(h2[:], w2_s[i][:], h1s[:],
                                     start=True, stop=True)
                    # x += h2 + b2 (bf16 residual carrier)
                    nc.vector.scalar_tensor_tensor(
                        out=x_ap, in0=h2[:], scalar=b2_s[i][:],
                        in1=x_ap, op0=ALU.add, op1=ALU.add)

                    if i < nb - 1:
                        # bridge: transpose x_new into 4 stacked 128x128
                        # node-major tiles, accumulate spectral delta on PE
                        xnm = wk.tile([128, 4, 128], BF, tag="xnm",
                                      name=f"xnm{cI}")
                        if BRIDGE_XBAR:
                            nc.sync.dma_start_transpose(xnm[:], x_ap)
                        else:
                            hT = pmm.tile([128, 4, 128], BF, tag="mm",
                                          name=f"hT{cI}")
                            for t in range(4):
                                nc.tensor.transpose(
                                    hT[:, t, :],
                                    x_ap[:, t * 128:(t + 1) * 128],
                                    ident_bf[:])
                            nc.vector.tensor_copy(xnm[:], hT[:])
                        spec_dst = specA_ps if cI < nch // 2 else specB_ps
                        first = cI in (0, nch // 2)
                        last = cI in (nch // 2 - 1, nch - 1)
                        for t in range(4):
                            nc.tensor.matmul(
                                spec_dst,
                                xnm[:, t, :],
                                trio_t[cI][:, 2, t * 128:(t + 1) * 128],
                                start=(first and t == 0),
                                stop=(last and t == 3),
                                skip_group_check=True)
                        if cI == nch // 2 - 1:
                            spA = sm.tile([C, K], F32, tag="spA")
                            nc.vector.tensor_copy(spA[:], specA_ps)
                            nc.sync.dma_start(cc_iA[:], spA[:])
                            if USE_CC:
                                nc.gpsimd.collective_compute(
                                    "AllReduce", ALU.add,
                                    replica_groups=PAIRS[:ncores // 2],
                                    ins=[cc_iA.opt()], outs=[cc_oA.opt()])
                            else:
                                nc.sync.dma_start(cc_oA[:], spA[:])
                        elif cI == nch - 1:
                            spB = sm.tile([C, K], F32, tag="spB")
                            nc.vector.tensor_copy(spB[:], specB_ps)
                            nc.sync.dma_start(cc_iB[:], spB[:])
                            if USE_CC:
                                nc.gpsimd.collective_compute(
                                    "AllReduce", ALU.add,
                                    replica_groups=PAIRS[:ncores // 2],
                                    ins=[cc_iB.opt()], outs=[cc_oB.opt()])
                            else:
                                nc.sync.dma_start(cc_oB[:], spB[:])
                    else:
                        # output head
                        y = pmm.tile([3, CH], F32, tag="mm", name=f"y{cI}")
                        nc.tensor.matmul(y[:], wlast_s[:], x_ap,
                                         start=True, stop=True)
                        ysb = wk.tile([3, CH], F32, tag="y", name=f"ys{cI}")
                        nc.vector.tensor_scalar_add(ysb[:], y[:], blast_s[:])
                        nc.sync.dma_start(yT[:, cI * CH:(cI + 1) * CH], ysb[:])

                a1_prev = None
                for cI in range(nch + 1):
                    if cI < nch:
                        a1_cur = stage_a(cI)
                    if cI >= 1:
                        stage_b(cI - 1, a1_prev)
                    a1_prev = a1_cur

    nc.compile()
    return nc


_NC_CACHE = {}


def _get_nc():
    if "nc" not in _NC_CACHE:
        _NC_CACHE["nc"] = build_nc()
    return _NC_CACHE["nc"]


def kernel(**inputs):
    nc = _get_nc()
    in_maps = host_prep(inputs)
    res = run_bass_kernel_spmd(nc, in_maps, core_ids=list(range(NCORES)))
    out = np.empty((B, N, 3), np.float32)
    for b in range(B):
        for h in range(2):
            yT = res.results[2 * b + h]["yT"]
            out[b, h * NH:(h + 1) * NH] = yT[:, :NH].T
    return out


# revision 33
# speedup vs baseline: 1.7624x; 1.1784x over previous
"""DiffusionNet forward on 8 Trainium2 NeuronCores.

Strategy (v3)
-------------
B=4 samples, 2 cores per sample, each core owns half the mesh nodes
(20000, zero-padded to 20480).  All cross-node coupling flows through the
K=128 spectral bottleneck:

  * SpMM eliminated on device: gX = (G @ evecs) @ S, host precomputes
    GXe = G @ evecs once per sample (exact associativity).
  * Everything big is SBUF-resident for the whole kernel (no per-block
    re-streaming): x (bf16), ev (bf16, K-major), evm (bf16, node-major)
    and the gradient operators (fp8e4, x64 scaled, [k, j, n] layout with
    j in {gx, gy}).
  * The four spectral-stream matmuls per chunk (gX, gY, Br, Bi) are each
    ONE fp8 DoubleRow matmul contracting 256 = K x {x,y}: the pair dim
    holds the gx/gy interleave, so Br = Sre@gx - Sim@gy needs no PSUM
    accumulation (accumulating DoubleRow pairs crash the runtime).
  * The forward spectral transform of the NEXT block is fused into the
    sweep: after the residual update of a chunk, its x tiles are
    transposed and immediately accumulated into the spectral partial
    (PSUM), split into two halves so the pair AllReduce of the first
    half hides under the second half of the sweep.
  * Block 1's spectrum is precomputed on host (full-sample sum), so no
    standalone forward pass and no AllReduce before the first sweep.
  * Elementwise work is spread over DVE (m1, m2, residual), Act (Br/Bi
    evictions, tanh, relus, casts) and Pool/gpsimd (a1 = m1 + m2).
"""

import sys
import numpy as np
import ml_dtypes

for _p in ("/opt/trn_rl_repo", "/root/.axon_site/_ro/trn_rl_repo"):
    if _p not in sys.path:
        sys.path.append(_p)

import concourse.bass as bass
import concourse.bacc as bacc
import concourse.tile as tile
import concourse.mybir as mybir
from concourse.bass_utils import run_bass_kernel_spmd
from concourse.masks import make_identity

BF = mybir.dt.bfloat16
F32 = mybir.dt.float32
F8 = mybir.dt.float8e4
AF = mybir.ActivationFunctionType
ALU = mybir.AluOpType
DR = mybir.MatmulPerfMode.DoubleRow

B, N, E, K = 4, 40000, 240000, 128
C = 128
NB = 4          # diffusion blocks
NCORES = 8
NH = N // 2     # nodes per core (half sample)
CH = 512        # node chunk (matmul free dim)
NHP = 20480     # padded nodes per core: 40 chunks * 512
NCH = NHP // CH
PAIRS = [[0, 1], [2, 3], [4, 5], [6, 7]]
SG = 64.0       # fp8 scale on GXe/GYe; tanh un-scales by 1/SG^2
BRIDGE_XBAR = False  # xbar-DMA transpose bridge vs PE transpose + DVE evict
POOL_A1 = True       # a1 on gpsimd/Pool vs DVE
USE_CC = True        # pairwise AllReduce vs local-only (debug)

bf16 = ml_dtypes.bfloat16
f8e4 = ml_dtypes.float8_e4m3


# ----------------------------------------------------------------- host side

def _spmm_mat(rows, cols, vals, M):
    """(COO [N,N] with given pattern) @ M, dense M [N,k]. Pure numpy."""
    out = np.zeros((N, M.shape[1]), np.float32)
    perm = np.argsort(rows, kind="stable")
    contrib = (vals[:, None] * M[cols]).astype(np.float32)[perm]
    rs = rows[perm]
    uniq, starts = np.unique(rs, return_index=True)
    out[uniq] = np.add.reduceat(contrib, starts, axis=0)
    return out


def host_prep(inputs, nhp=NHP, nb=NB):
    """Build the 8 per-core input dicts."""
    x_in = np.asarray(inputs["x_in"], np.float32)
    mass = np.asarray(inputs["mass"], np.float32)
    evals = np.asarray(inputs["evals"], np.float32)
    evecs = np.asarray(inputs["evecs"], np.float32)
    rows = np.asarray(inputs["rows"])
    cols = np.asarray(inputs["cols"])
    gX_vals = np.asarray(inputs["gradX_vals"], np.float32)
    gY_vals = np.asarray(inputs["gradY_vals"], np.float32)
    w_first = np.asarray(inputs["w_first"], np.float32)
    b_first = np.asarray(inputs["b_first"], np.float32)
    diff_time = np.asarray(inputs["diff_time"], np.float32)
    A_re = np.asarray(inputs["A_re"], np.float32)
    A_im = np.asarray(inputs["A_im"], np.float32)
    mlp_w0 = np.asarray(inputs["mlp_w0"], np.float32)
    w1 = np.asarray(inputs["mlp_w1"], np.float32)
    w2 = np.asarray(inputs["mlp_w2"], np.float32)
    b0 = np.asarray(inputs["mlp_b0"], np.float32)
    b1 = np.asarray(inputs["mlp_b1"], np.float32)
    b2 = np.asarray(inputs["mlp_b2"], np.float32)
    w_last = np.asarray(inputs["w_last"], np.float32)
    b_last = np.asarray(inputs["b_last"], np.float32)

    nh = NH

    shared = dict(
        Are=A_re[:nb].astype(bf16),
        Aim=A_im[:nb].astype(bf16),
        w0a=np.ascontiguousarray(mlp_w0[:nb, 0:C]).astype(bf16),
        w0b=np.ascontiguousarray(mlp_w0[:nb, C:2 * C]).astype(bf16),
        w0c=np.ascontiguousarray(mlp_w0[:nb, 2 * C:3 * C]).astype(bf16),
        w1=w1[:nb].astype(bf16),
        w2=w2[:nb].astype(bf16),
        b0=b0[:nb].reshape(nb, C, 1),
        b1=b1[:nb].reshape(nb, C, 1),
        b2=b2[:nb].reshape(nb, C, 1),
        wlast=w_last.astype(bf16),
        blast=b_last.reshape(3, 1),
    )

    in_maps = []
    for b in range(B):
        ev = evecs[b]
        evm_full = ev * mass[b][:, None]
        GXe = _spmm_mat(rows, cols, gX_vals[b], ev)
        GYe = _spmm_mat(rows, cols, gY_vals[b], ev)
        x0_full = x_in[b] @ w_first + b_first
        # spec for block 0, full-sample sum (both halves): [C, K]
        spec1 = (x0_full.T @ evm_full).astype(np.float32)
        # coefs[i][c,k] = exp(-evals[k] * diff_time[i][c])
        coefs = np.exp(-evals[b][None, None, :]
                       * diff_time[:nb, :, None]).astype(np.float32)
        for h in range(2):
            sl = slice(h * nh, (h + 1) * nh)

            def padT(M):  # [nh, D] -> [D, nhp]
                out = np.zeros((M.shape[1], nhp), np.float32)
                out[:, :nh] = M[sl].T
                return out

            # trio [nch, 128, 3, 512] bf16: 0 = x0, 1 = ev (K-major),
            # 2 = evm node-major tiles (t, k)
            x0c = padT(x0_full).reshape(C, NCH, CH).transpose(1, 0, 2)
            evc = padT(ev).reshape(K, NCH, CH).transpose(1, 0, 2)
            evmP = np.zeros((nhp, K), np.float32)
            evmP[:nh] = evm_full[sl]
            evmc = evmP.reshape(NCH, 4, 128, K).transpose(0, 2, 1, 3) \
                       .reshape(NCH, 128, CH)
            trio = np.stack([x0c, evc, evmc], axis=2).astype(bf16)

            # gxy [nch, 128(k), 2(j), 512(n)] fp8: j=0 gx, j=1 gy
            def kpack(Gm):  # [nh, K] -> [nch, 128, 512]
                GT = np.clip(padT(Gm) * SG, -240.0, 240.0)  # [K, nhp]
                return GT.reshape(K, NCH, CH).transpose(1, 0, 2)

            gxy = np.stack([kpack(GXe), kpack(GYe)], axis=2).astype(f8e4)

            in_maps.append(dict(
                trio=trio,
                gxy=np.ascontiguousarray(gxy),
                spec1=spec1,
                coefs=coefs,
                **shared,
            ))
    return in_maps


# --------------------------------------------------------------- device side

def build_nc(nb=NB, nch=NCH, ncores=NCORES):
    nhp = nch * CH
    nc = bacc.Bacc("TRN2", target_bir_lowering=False, debug=False,
                   enable_asserts=False, num_devices=ncores)

    trio = nc.dram_tensor("trio", [nch, 128, 3, CH], BF, kind="ExternalInput")
    gxy = nc.dram_tensor("gxy", [nch, 128, 2, CH], F8, kind="ExternalInput")
    spec1 = nc.dram_tensor("spec1", [C, K], F32, kind="ExternalInput")
    coefs = nc.dram_tensor("coefs", [nb, C, K], F32, kind="ExternalInput")
    Are = nc.dram_tensor("Are", [nb, C, C], BF, kind="ExternalInput")
    Aim = nc.dram_tensor("Aim", [nb, C, C], BF, kind="ExternalInput")
    w0a = nc.dram_tensor("w0a", [nb, C, C], BF, kind="ExternalInput")
    w0b = nc.dram_tensor("w0b", [nb, C, C], BF, kind="ExternalInput")
    w0c = nc.dram_tensor("w0c", [nb, C, C], BF, kind="ExternalInput")
    w1 = nc.dram_tensor("w1", [nb, C, C], BF, kind="ExternalInput")
    w2 = nc.dram_tensor("w2", [nb, C, C], BF, kind="ExternalInput")
    b0 = nc.dram_tensor("b0", [nb, C, 1], F32, kind="ExternalInput")
    b1 = nc.dram_tensor("b1", [nb, C, 1], F32, kind="ExternalInput")
    b2 = nc.dram_tensor("b2", [nb, C, 1], F32, kind="ExternalInput")
    wlast = nc.dram_tensor("wlast", [C, 3], BF, kind="ExternalInput")
    blast = nc.dram_tensor("blast", [3, 1], F32, kind="ExternalInput")
    yT = nc.dram_tensor("yT", [3, nhp], F32, kind="ExternalOutput")

    with tile.TileContext(nc) as tc:
        with (
            tc.tile_pool(name="consts", bufs=1) as consts,
            tc.tile_pool(name="res", bufs=1) as res,
            tc.tile_pool(name="wk", bufs=2) as wk,
            tc.tile_pool(name="sm", bufs=2) as sm,
            tc.tile_pool(name="pmm", bufs=5, space="PSUM") as pmm,
            tc.tile_pool(name="ph0", bufs=2, space="PSUM") as ph0,
            tc.tile_pool(name="psm", bufs=1, space="PSUM") as psm,
            tc.tile_pool(name="dram", bufs=2, space="DRAM") as dram,
        ):
            ident_bf = consts.tile([128, 128], BF, tag="identb")
            make_identity(nc, ident_bf[:])

            def cload(src, shape, dt, tag):
                t = consts.tile(shape, dt, tag=tag)
                nc.sync.dma_start(t[:], src)
                return t

            coefs_s = [cload(coefs[i], [C, K], F32, f"cf{i}") for i in range(nb)]
            Are_s = [cload(Are[i], [C, C], BF, f"Are{i}") for i in range(nb)]
            Aim_s = [cload(Aim[i], [C, C], BF, f"Aim{i}") for i in range(nb)]
            w0a_s = [cload(w0a[i], [C, C], BF, f"w0a{i}") for i in range(nb)]
            w0b_s = [cload(w0b[i], [C, C], BF, f"w0b{i}") for i in range(nb)]
            w0c_s = [cload(w0c[i], [C, C], BF, f"w0c{i}") for i in range(nb)]
            w1_s = [cload(w1[i], [C, C], BF, f"w1{i}") for i in range(nb)]
            w2_s = [cload(w2[i], [C, C], BF, f"w2{i}") for i in range(nb)]
            b0_s = [cload(b0[i], [C, 1], F32, f"b0{i}") for i in range(nb)]
            b1_s = [cload(b1[i], [C, 1], F32, f"b1{i}") for i in range(nb)]
            b2_s = [cload(b2[i], [C, 1], F32, f"b2{i}") for i in range(nb)]
            wlast_s = cload(wlast[:], [C, 3], BF, "wlast")
            blast_s = cload(blast[:], [3, 1], F32, "blast")
            spec1_s = cload(spec1[:], [C, K], F32, "spec1")

            # resident per-chunk tiles, streamed in once
            trio_t, gxy_t = [], []
            for cI in range(nch):
                t = res.tile([128, 3, CH], BF, tag=f"trio{cI}")
                nc.sync.dma_start(t[:], trio[cI])
                trio_t.append(t)
                g = res.tile([128, 2, CH], F8, tag=f"gxy{cI}")
                nc.sync.dma_start(g[:], gxy[cI])
                gxy_t.append(g)

            # PSUM scratch banks for small accumulators, manually packed
            s1 = psm.tile([128, CH], F32, tag="s1")
            specA_ps = s1[:, 0:128]
            specB_ps = s1[:, 128:256]
            sw0b_ps = s1[:, 256:384]                      # [K, C] f32

            if USE_CC:
                # collective warmup (ring spin-up off the critical path)
                warm_sb = sm.tile([C, K], F32, tag="warm", bufs=1)
                nc.gpsimd.memset(warm_sb[:], 0.0)
                cc_wi = dram.tile([C, K], F32, tag="cwi", bufs=1)
                cc_wo = dram.tile([C, K], F32, tag="cwo", bufs=1)
                nc.sync.dma_start(cc_wi[:], warm_sb[:])
                nc.gpsimd.collective_compute(
                    "AllReduce", ALU.add, replica_groups=PAIRS[:ncores // 2],
                    ins=[cc_wi.opt()], outs=[cc_wo.opt()])

            for i in range(nb):
                # ---- smalls: DoubleRow W matrices from the block spectrum
                #      WgX=[S|0] WgY=[0|S] WBr=[Sre|-Sim] WBi=[Sim|Sre],
                #      each [k, j, c] fp8 with j the gx/gy pair dim ----
                if i == 0:
                    spec_f = spec1_s
                else:
                    sA = sm.tile([C, K], F32, tag="sA")
                    nc.sync.dma_start(sA[:], cc_oA[:])
                    sB = sm.tile([C, K], F32, tag="sB")
                    nc.sync.dma_start(sB[:], cc_oB[:])
                    spec_f = sm.tile([C, K], F32, tag="spec")
                    nc.vector.tensor_add(spec_f[:], sA[:], sB[:])

                s2 = pmm.tile([128, CH], F32, tag="mm", name=f"sml{i}")
                sretT_ps = s2[:, 0:128]                   # Sre^T [C, K] f32
                simtT_ps = s2[:, 128:256]                 # Sim^T [C, K] f32
                bfh = s2[:, 256:384].bitcast(BF)          # [128, 256] bf16
                S_ps = bfh[:, 0:128]                      # S [K, C] bf16
                Sre_ps = bfh[:, 128:256]                  # Sre [K, C] bf16
                Sim_ps = s2[:, 384:448].bitcast(BF)       # Sim [K, C] bf16
                stf_bf = sm.tile([C, K], BF, tag="stfb")
                nc.vector.tensor_mul(stf_bf[:], spec_f[:], coefs_s[i][:])
                # S = (S^T)^T first: WgX/WgY unblock the sweep
                nc.tensor.matmul(S_ps, stf_bf[:], ident_bf[:],
                                 is_transpose=True, skip_group_check=True)
                wgx = sm.tile([128, 2, 128], F8, tag="wgx")
                nc.scalar.activation(wgx[:, 0, :], S_ps, AF.Copy)
                nc.scalar.activation(wgx[:, 1, :], S_ps, AF.Copy, scale=0.0)
                wgy = sm.tile([128, 2, 128], F8, tag="wgy")
                nc.scalar.activation(wgy[:, 0, :], S_ps, AF.Copy, scale=0.0)
                nc.scalar.activation(wgy[:, 1, :], S_ps, AF.Copy)

                nc.tensor.matmul(sretT_ps, Are_s[i][:], stf_bf[:],
                                 start=True, stop=True, skip_group_check=True)
                nc.tensor.matmul(simtT_ps, Aim_s[i][:], stf_bf[:],
                                 start=True, stop=True, skip_group_check=True)
                sret_bf = sm.tile([C, K], BF, tag="sreb")
                nc.scalar.activation(sret_bf[:], sretT_ps, AF.Copy)
                simt_bf = sm.tile([C, K], BF, tag="simb")
                nc.scalar.activation(simt_bf[:], simtT_ps, AF.Copy)
                nc.tensor.matmul(Sre_ps, sret_bf[:], ident_bf[:],
                                 is_transpose=True, skip_group_check=True)
                nc.tensor.matmul(Sim_ps, simt_bf[:], ident_bf[:],
                                 is_transpose=True, skip_group_check=True)
                wbr = sm.tile([128, 2, 128], F8, tag="wbr")
                nc.scalar.activation(wbr[:, 0, :], Sre_ps, AF.Copy)
                nc.scalar.activation(wbr[:, 1, :], Sim_ps, AF.Copy, scale=-1.0)
                wbi = sm.tile([128, 2, 128], F8, tag="wbi")
                nc.scalar.activation(wbi[:, 0, :], Sim_ps, AF.Copy)
                nc.scalar.activation(wbi[:, 1, :], Sre_ps, AF.Copy)

                nc.tensor.matmul(sw0b_ps, stf_bf[:], w0b_s[i][:],
                                 start=True, stop=True, skip_group_check=True)
                sw0b_bf = sm.tile([K, C], BF, tag="sw0b")
                nc.scalar.activation(sw0b_bf[:], sw0b_ps, AF.Copy)

                if i < nb - 1:
                    cc_iA = dram.tile([C, K], F32, tag="ciA")
                    cc_oA = dram.tile([C, K], F32, tag="coA")
                    cc_iB = dram.tile([C, K], F32, tag="ciB")
                    cc_oB = dram.tile([C, K], F32, tag="coB")

                # ---- fused sweep over node chunks, software-pipelined:
                #      stage A(k) = quad + gX/gY evictions + m1/m2/a1,
                #      stage B(k) = tanh + MLP + relus + residual + bridge.
                #      Emitting A(k+1) before B(k) keeps the PE queue from
                #      blocking on the gf dependency. ----
                def stage_1(cI):
                    g_ap = gxy_t[cI][:]
                    gX = pmm.tile([C, CH], F32, tag="mm", name=f"gX{cI}")
                    nc.tensor.matmul(gX[:], wgx[:], g_ap,
                                     start=True, stop=True, perf_mode=DR)
                    gY = pmm.tile([C, CH], F32, tag="mm", name=f"gY{cI}")
                    nc.tensor.matmul(gY[:], wgy[:], g_ap,
                                     start=True, stop=True, perf_mode=DR)
                    gx_sb = wk.tile([C, CH], BF, tag="gxs", name=f"gxs{cI}")
                    nc.scalar.activation(gx_sb[:], gX[:], AF.Copy)
                    gy_sb = wk.tile([C, CH], BF, tag="gys", name=f"gys{cI}")
                    nc.scalar.activation(gy_sb[:], gY[:], AF.Copy)
                    Br = pmm.tile([C, CH], F32, tag="mm", name=f"Br{cI}")
                    nc.tensor.matmul(Br[:], wbr[:], g_ap,
                                     start=True, stop=True, perf_mode=DR)
                    Bi = pmm.tile([C, CH], F32, tag="mm", name=f"Bi{cI}")
                    nc.tensor.matmul(Bi[:], wbi[:], g_ap,
                                     start=True, stop=True, perf_mode=DR)
                    m1 = wk.tile([C, CH], BF, tag="m1", name=f"m1_{cI}")
                    nc.vector.tensor_mul(m1[:], Br[:], gx_sb[:])
                    m2 = wk.tile([C, CH], BF, tag="m2", name=f"m2_{cI}")
                    nc.vector.tensor_mul(m2[:], Bi[:], gy_sb[:])
                    a1 = wk.tile([C, CH], BF, tag="a1", name=f"a1_{cI}")
                    if POOL_A1:
                        nc.gpsimd.tensor_add(a1[:], m1[:], m2[:])
                    else:
                        nc.vector.tensor_add(a1[:], m1[:], m2[:])
                    return a1

                def stage_2(cI):
                    x_ap = trio_t[cI][:, 0, :]
                    ev_ap = trio_t[cI][:, 1, :]
                    h0 = ph0.tile([C, CH], F32, tag="h0", name=f"h0_{cI}")
                    nc.tensor.matmul(h0[:], w0a_s[i][:], x_ap,
                                     start=True, stop=False)
                    nc.tensor.matmul(h0[:], sw0b_bf[:], ev_ap,
                                     start=False, stop=False)
                    return h0

                def stage_3(cI, a1, h0):
                    x_ap = trio_t[cI][:, 0, :]
                    gf = wk.tile([C, CH], BF, tag="gf", name=f"gf{cI}")
                    nc.scalar.activation(gf[:], a1[:], AF.Tanh,
                                         scale=1.0 / (SG * SG))
                    nc.tensor.matmul(h0[:], w0c_s[i][:], gf[:],
                                     start=False, stop=True)
                    h0s = wk.tile([C, CH], BF, tag="h0s", name=f"h0s{cI}")
                    nc.scalar.activation(h0s[:], h0[:], AF.Relu,
                                         bias=b0_s[i][:])
                    h1 = pmm.tile([C, CH], F32, tag="mm", name=f"h1_{cI}")
                    nc.tensor.matmul(h1[:], w1_s[i][:], h0s[:],
                                     start=True, stop=True)
                    h1s = wk.tile([C, CH], BF, tag="h1s", name=f"h1s{cI}")
                    nc.vector.tensor_scalar(h1s[:], h1[:], b1_s[i][:], 0.0,
                                            ALU.add, ALU.max)
                    h2 = pmm.tile([C, CH], F32, tag="mm", name=f"h2_{cI}")
                    nc.tensor.matmul(h2[:], w2_s[i][:], h1s[:],
                                     start=True, stop=True)
                    # x += h2 + b2 (bf16 residual carrier)
                    nc.vector.scalar_tensor_tensor(
                        out=x_ap, in0=h2[:], scalar=b2_s[i][:],
                        in1=x_ap, op0=ALU.add, op1=ALU.add)

                    if i < nb - 1:
                        # bridge: transpose x_new into 4 stacked 128x128
                        # node-major tiles, accumulate spectral delta on PE
                        xnm = wk.tile([128, 4, 128], BF, tag="xnm",
                                      name=f"xnm{cI}")
                        if BRIDGE_XBAR:
                            nc.sync.dma_start_transpose(xnm[:], x_ap)
                        else:
                            hT = pmm.tile([128, 4, 128], BF, tag="mm",
                                          name=f"hT{cI}")
                            for t in range(4):
                                nc.tensor.transpose(
                                    hT[:, t, :],
                                    x_ap[:, t * 128:(t + 1) * 128],
                                    ident_bf[:])
                            nc.vector.tensor_copy(xnm[:], hT[:])
                        spec_dst = specA_ps if cI < nch // 2 else specB_ps
                        first = cI in (0, nch // 2)
                        last = cI in (nch // 2 - 1, nch - 1)
                        for t in range(4):
                            nc.tensor.matmul(
                                spec_dst,
                                xnm[:, t, :],
                                trio_t[cI][:, 2, t * 128:(t + 1) * 128],
                                start=(first and t == 0),
                                stop=(last and t == 3),
                                skip_group_check=True)
                        if cI == nch // 2 - 1:
                            spA = sm.tile([C, K], F32, tag="spA")
                            nc.vector.tensor_copy(spA[:], specA_ps)
                            nc.sync.dma_start(cc_iA[:], spA[:])
                            if USE_CC:
                                nc.gpsimd.collective_compute(
                                    "AllReduce", ALU.add,
                                    replica_groups=PAIRS[:ncores // 2],
                                    ins=[cc_iA.opt()], outs=[cc_oA.opt()])
                            else:
                                nc.sync.dma_start(cc_oA[:], spA[:])
                        elif cI == nch - 1:
                            spB = sm.tile([C, K], F32, tag="spB")
                            nc.vector.tensor_copy(spB[:], specB_ps)
                            nc.sync.dma_start(cc_iB[:], spB[:])
                            if USE_CC:
                                nc.gpsimd.collective_compute(
                                    "AllReduce", ALU.add,
                                    replica_groups=PAIRS[:ncores // 2],
                                    ins=[cc_iB.opt()], outs=[cc_oB.opt()])
                            else:
                                nc.sync.dma_start(cc_oB[:], spB[:])
                    else:
                        # output head
                        y = pmm.tile([3, CH], F32, tag="mm", name=f"y{cI}")
                        nc.tensor.matmul(y[:], wlast_s[:], x_ap,
                                         start=True, stop=True)
                        ysb = wk.tile([3, CH], F32, tag="y", name=f"ys{cI}")
                        nc.vector.tensor_scalar_add(ysb[:], y[:], blast_s[:])
                        nc.sync.dma_start(yT[:, cI * CH:(cI + 1) * CH], ysb[:])

                carry = None
                for cI in range(nch + 1):
                    if cI < nch:
                        a1_c = stage_1(cI)
                        h0_c = stage_2(cI)
                    if cI >= 1:
                        stage_3(cI - 1, *carry)
                    carry = (a1_c, h0_c)

    nc.compile()
    return nc


_NC_CACHE = {}


def _get_nc():
    if "nc" not in _NC_CACHE:
        _NC_CACHE["nc"] = build_nc()
    return _NC_CACHE["nc"]


def kernel(**inputs):
    nc = _get_nc()
    in_maps = host_prep(inputs)
    res = run_bass_kernel_spmd(nc, in_maps, core_ids=list(range(NCORES)))
    out = np.empty((B, N, 3), np.float32)
    for b in range(B):
        for h in range(2):
            yT = res.results[2 * b + h]["yT"]
            out[b, h * NH:(h + 1) * NH] = yT[:, :NH].T
    return out


# revision 34
# speedup vs baseline: 1.9724x; 1.1191x over previous
"""DiffusionNet forward on 8 Trainium2 NeuronCores.

Strategy (v3)
-------------
B=4 samples, 2 cores per sample, each core owns half the mesh nodes
(20000, zero-padded to 20480).  All cross-node coupling flows through the
K=128 spectral bottleneck:

  * SpMM eliminated on device: gX = (G @ evecs) @ S, host precomputes
    GXe = G @ evecs once per sample (exact associativity).
  * Everything big is SBUF-resident for the whole kernel (no per-block
    re-streaming): x (bf16), ev (bf16, K-major), evm (bf16, node-major)
    and the gradient operators (fp8e4, x64 scaled, [k, j, n] layout with
    j in {gx, gy}).
  * The four spectral-stream matmuls per chunk (gX, gY, Br, Bi) are each
    ONE fp8 DoubleRow matmul contracting 256 = K x {x,y}: the pair dim
    holds the gx/gy interleave, so Br = Sre@gx - Sim@gy needs no PSUM
    accumulation (accumulating DoubleRow pairs crash the runtime).
  * The forward spectral transform of the NEXT block is fused into the
    sweep: after the residual update of a chunk, its x tiles are
    transposed and immediately accumulated into the spectral partial
    (PSUM), split into two halves so the pair AllReduce of the first
    half hides under the second half of the sweep.
  * Block 1's spectrum is precomputed on host (full-sample sum), so no
    standalone forward pass and no AllReduce before the first sweep.
  * Elementwise work is spread over DVE (m1, m2, residual), Act (Br/Bi
    evictions, tanh, relus, casts) and Pool/gpsimd (a1 = m1 + m2).
"""

import sys
import numpy as np
import ml_dtypes

for _p in ("/opt/trn_rl_repo", "/root/.axon_site/_ro/trn_rl_repo"):
    if _p not in sys.path:
        sys.path.append(_p)

import concourse.bass as bass
import concourse.bacc as bacc
import concourse.tile as tile
import concourse.mybir as mybir
from concourse.bass_utils import run_bass_kernel_spmd
from concourse.masks import make_identity

BF = mybir.dt.bfloat16
F32 = mybir.dt.float32
F8 = mybir.dt.float8e4
AF = mybir.ActivationFunctionType
ALU = mybir.AluOpType
DR = mybir.MatmulPerfMode.DoubleRow

B, N, E, K = 4, 40000, 240000, 128
C = 128
NB = 4          # diffusion blocks
NCORES = 8
NH = N // 2     # nodes per core (half sample)
CH = 512        # node chunk (matmul free dim)
NHP = 20480     # padded nodes per core: 40 chunks * 512
NCH = NHP // CH
PAIRS = [[0, 1], [2, 3], [4, 5], [6, 7]]
SG = 64.0       # fp8 scale on GXe/GYe; tanh un-scales by 1/SG^2
BRIDGE_XBAR = True   # xbar-DMA transpose bridge vs PE transpose + DVE evict
POOL_A1 = True       # a1 on gpsimd/Pool vs DVE
USE_CC = True        # pairwise AllReduce vs local-only (debug)

bf16 = ml_dtypes.bfloat16
f8e4 = ml_dtypes.float8_e4m3


# ----------------------------------------------------------------- host side

def _spmm_mat(rows, cols, vals, M):
    """(COO [N,N] with given pattern) @ M, dense M [N,k]. Pure numpy."""
    out = np.zeros((N, M.shape[1]), np.float32)
    perm = np.argsort(rows, kind="stable")
    contrib = (vals[:, None] * M[cols]).astype(np.float32)[perm]
    rs = rows[perm]
    uniq, starts = np.unique(rs, return_index=True)
    out[uniq] = np.add.reduceat(contrib, starts, axis=0)
    return out


def host_prep(inputs, nhp=NHP, nb=NB):
    """Build the 8 per-core input dicts."""
    x_in = np.asarray(inputs["x_in"], np.float32)
    mass = np.asarray(inputs["mass"], np.float32)
    evals = np.asarray(inputs["evals"], np.float32)
    evecs = np.asarray(inputs["evecs"], np.float32)
    rows = np.asarray(inputs["rows"])
    cols = np.asarray(inputs["cols"])
    gX_vals = np.asarray(inputs["gradX_vals"], np.float32)
    gY_vals = np.asarray(inputs["gradY_vals"], np.float32)
    w_first = np.asarray(inputs["w_first"], np.float32)
    b_first = np.asarray(inputs["b_first"], np.float32)
    diff_time = np.asarray(inputs["diff_time"], np.float32)
    A_re = np.asarray(inputs["A_re"], np.float32)
    A_im = np.asarray(inputs["A_im"], np.float32)
    mlp_w0 = np.asarray(inputs["mlp_w0"], np.float32)
    w1 = np.asarray(inputs["mlp_w1"], np.float32)
    w2 = np.asarray(inputs["mlp_w2"], np.float32)
    b0 = np.asarray(inputs["mlp_b0"], np.float32)
    b1 = np.asarray(inputs["mlp_b1"], np.float32)
    b2 = np.asarray(inputs["mlp_b2"], np.float32)
    w_last = np.asarray(inputs["w_last"], np.float32)
    b_last = np.asarray(inputs["b_last"], np.float32)

    nh = NH

    shared = dict(
        Are=A_re[:nb].astype(bf16),
        Aim=A_im[:nb].astype(bf16),
        w0a=np.ascontiguousarray(mlp_w0[:nb, 0:C]).astype(bf16),
        w0b=np.ascontiguousarray(mlp_w0[:nb, C:2 * C]).astype(bf16),
        w0c=np.ascontiguousarray(mlp_w0[:nb, 2 * C:3 * C]).astype(bf16),
        w1=w1[:nb].astype(bf16),
        w2=w2[:nb].astype(bf16),
        b0=b0[:nb].reshape(nb, C, 1),
        b1=b1[:nb].reshape(nb, C, 1),
        b2=b2[:nb].reshape(nb, C, 1),
        wlast=w_last.astype(bf16),
        blast=b_last.reshape(3, 1),
    )

    in_maps = []
    for b in range(B):
        ev = evecs[b]
        evm_full = ev * mass[b][:, None]
        GXe = _spmm_mat(rows, cols, gX_vals[b], ev)
        GYe = _spmm_mat(rows, cols, gY_vals[b], ev)
        x0_full = x_in[b] @ w_first + b_first
        # spec for block 0, full-sample sum (both halves): [C, K]
        spec1 = (x0_full.T @ evm_full).astype(np.float32)
        # coefs[i][c,k] = exp(-evals[k] * diff_time[i][c])
        coefs = np.exp(-evals[b][None, None, :]
                       * diff_time[:nb, :, None]).astype(np.float32)
        for h in range(2):
            sl = slice(h * nh, (h + 1) * nh)

            def padT(M):  # [nh, D] -> [D, nhp]
                out = np.zeros((M.shape[1], nhp), np.float32)
                out[:, :nh] = M[sl].T
                return out

            # trio [nch, 128, 3, 512] bf16: 0 = x0, 1 = ev (K-major),
            # 2 = evm node-major tiles (t, k)
            x0c = padT(x0_full).reshape(C, NCH, CH).transpose(1, 0, 2)
            evc = padT(ev).reshape(K, NCH, CH).transpose(1, 0, 2)
            evmP = np.zeros((nhp, K), np.float32)
            evmP[:nh] = evm_full[sl]
            evmc = evmP.reshape(NCH, 4, 128, K).transpose(0, 2, 1, 3) \
                       .reshape(NCH, 128, CH)
            trio = np.stack([x0c, evc, evmc], axis=2).astype(bf16)

            # gxy [nch, 128(k), 2(j), 512(n)] fp8: j=0 gx, j=1 gy
            def kpack(Gm):  # [nh, K] -> [nch, 128, 512]
                GT = np.clip(padT(Gm) * SG, -240.0, 240.0)  # [K, nhp]
                return GT.reshape(K, NCH, CH).transpose(1, 0, 2)

            gxy = np.stack([kpack(GXe), kpack(GYe)], axis=2).astype(f8e4)

            in_maps.append(dict(
                trio=trio,
                gxy=np.ascontiguousarray(gxy),
                spec1=spec1,
                coefs=coefs,
                **shared,
            ))
    return in_maps


# --------------------------------------------------------------- device side

def build_nc(nb=NB, nch=NCH, ncores=NCORES):
    nhp = nch * CH
    nc = bacc.Bacc("TRN2", target_bir_lowering=False, debug=False,
                   enable_asserts=False, num_devices=ncores)

    trio = nc.dram_tensor("trio", [nch, 128, 3, CH], BF, kind="ExternalInput")
    gxy = nc.dram_tensor("gxy", [nch, 128, 2, CH], F8, kind="ExternalInput")
    spec1 = nc.dram_tensor("spec1", [C, K], F32, kind="ExternalInput")
    coefs = nc.dram_tensor("coefs", [nb, C, K], F32, kind="ExternalInput")
    Are = nc.dram_tensor("Are", [nb, C, C], BF, kind="ExternalInput")
    Aim = nc.dram_tensor("Aim", [nb, C, C], BF, kind="ExternalInput")
    w0a = nc.dram_tensor("w0a", [nb, C, C], BF, kind="ExternalInput")
    w0b = nc.dram_tensor("w0b", [nb, C, C], BF, kind="ExternalInput")
    w0c = nc.dram_tensor("w0c", [nb, C, C], BF, kind="ExternalInput")
    w1 = nc.dram_tensor("w1", [nb, C, C], BF, kind="ExternalInput")
    w2 = nc.dram_tensor("w2", [nb, C, C], BF, kind="ExternalInput")
    b0 = nc.dram_tensor("b0", [nb, C, 1], F32, kind="ExternalInput")
    b1 = nc.dram_tensor("b1", [nb, C, 1], F32, kind="ExternalInput")
    b2 = nc.dram_tensor("b2", [nb, C, 1], F32, kind="ExternalInput")
    wlast = nc.dram_tensor("wlast", [C, 3], BF, kind="ExternalInput")
    blast = nc.dram_tensor("blast", [3, 1], F32, kind="ExternalInput")
    yT = nc.dram_tensor("yT", [3, nhp], F32, kind="ExternalOutput")

    with tile.TileContext(nc) as tc:
        with (
            tc.tile_pool(name="consts", bufs=1) as consts,
            tc.tile_pool(name="res", bufs=1) as res,
            tc.tile_pool(name="wk", bufs=2) as wk,
            tc.tile_pool(name="sm", bufs=2) as sm,
            tc.tile_pool(name="pmm", bufs=5, space="PSUM") as pmm,
            tc.tile_pool(name="ph0", bufs=2, space="PSUM") as ph0,
            tc.tile_pool(name="psm", bufs=1, space="PSUM") as psm,
            tc.tile_pool(name="dram", bufs=2, space="DRAM") as dram,
        ):
            ident_bf = consts.tile([128, 128], BF, tag="identb")
            make_identity(nc, ident_bf[:])

            def cload(src, shape, dt, tag):
                t = consts.tile(shape, dt, tag=tag)
                nc.sync.dma_start(t[:], src)
                return t

            coefs_s = [cload(coefs[i], [C, K], F32, f"cf{i}") for i in range(nb)]
            Are_s = [cload(Are[i], [C, C], BF, f"Are{i}") for i in range(nb)]
            Aim_s = [cload(Aim[i], [C, C], BF, f"Aim{i}") for i in range(nb)]
            w0a_s = [cload(w0a[i], [C, C], BF, f"w0a{i}") for i in range(nb)]
            w0b_s = [cload(w0b[i], [C, C], BF, f"w0b{i}") for i in range(nb)]
            w0c_s = [cload(w0c[i], [C, C], BF, f"w0c{i}") for i in range(nb)]
            w1_s = [cload(w1[i], [C, C], BF, f"w1{i}") for i in range(nb)]
            w2_s = [cload(w2[i], [C, C], BF, f"w2{i}") for i in range(nb)]
            b0_s = [cload(b0[i], [C, 1], F32, f"b0{i}") for i in range(nb)]
            b1_s = [cload(b1[i], [C, 1], F32, f"b1{i}") for i in range(nb)]
            b2_s = [cload(b2[i], [C, 1], F32, f"b2{i}") for i in range(nb)]
            wlast_s = cload(wlast[:], [C, 3], BF, "wlast")
            blast_s = cload(blast[:], [3, 1], F32, "blast")
            spec1_s = cload(spec1[:], [C, K], F32, "spec1")

            # resident per-chunk tiles, streamed in once
            trio_t, gxy_t = [], []
            for cI in range(nch):
                t = res.tile([128, 3, CH], BF, tag=f"trio{cI}")
                nc.sync.dma_start(t[:], trio[cI])
                trio_t.append(t)
                g = res.tile([128, 2, CH], F8, tag=f"gxy{cI}")
                nc.sync.dma_start(g[:], gxy[cI])
                gxy_t.append(g)

            # PSUM scratch banks for small accumulators, manually packed
            s1 = psm.tile([128, CH], F32, tag="s1")
            specA_ps = s1[:, 0:128]
            specB_ps = s1[:, 128:256]
            sw0b_ps = s1[:, 256:384]                      # [K, C] f32

            if USE_CC:
                # collective warmup (ring spin-up off the critical path)
                warm_sb = sm.tile([C, K], F32, tag="warm", bufs=1)
                nc.gpsimd.memset(warm_sb[:], 0.0)
                cc_wi = dram.tile([C, K], F32, tag="cwi", bufs=1)
                cc_wo = dram.tile([C, K], F32, tag="cwo", bufs=1)
                nc.sync.dma_start(cc_wi[:], warm_sb[:])
                nc.gpsimd.collective_compute(
                    "AllReduce", ALU.add, replica_groups=PAIRS[:ncores // 2],
                    ins=[cc_wi.opt()], outs=[cc_wo.opt()])

            for i in range(nb):
                # ---- smalls: DoubleRow W matrices from the block spectrum
                #      WgX=[S|0] WgY=[0|S] WBr=[Sre|-Sim] WBi=[Sim|Sre],
                #      each [k, j, c] fp8 with j the gx/gy pair dim ----
                if i == 0:
                    spec_f = spec1_s
                else:
                    sA = sm.tile([C, K], F32, tag="sA")
                    nc.sync.dma_start(sA[:], cc_oA[:])
                    sB = sm.tile([C, K], F32, tag="sB")
                    nc.sync.dma_start(sB[:], cc_oB[:])
                    spec_f = sm.tile([C, K], F32, tag="spec")
                    nc.vector.tensor_add(spec_f[:], sA[:], sB[:])

                s2 = pmm.tile([128, CH], F32, tag="mm", name=f"sml{i}")
                sretT_ps = s2[:, 0:128]                   # Sre^T [C, K] f32
                simtT_ps = s2[:, 128:256]                 # Sim^T [C, K] f32
                bfh = s2[:, 256:384].bitcast(BF)          # [128, 256] bf16
                S_ps = bfh[:, 0:128]                      # S [K, C] bf16
                Sre_ps = bfh[:, 128:256]                  # Sre [K, C] bf16
                Sim_ps = s2[:, 384:448].bitcast(BF)       # Sim [K, C] bf16
                stf_bf = sm.tile([C, K], BF, tag="stfb")
                nc.vector.tensor_mul(stf_bf[:], spec_f[:], coefs_s[i][:])
                # S = (S^T)^T first: WgX/WgY unblock the sweep
                nc.tensor.matmul(S_ps, stf_bf[:], ident_bf[:],
                                 is_transpose=True, skip_group_check=True)
                wgx = sm.tile([128, 2, 128], F8, tag="wgx")
                nc.scalar.activation(wgx[:, 0, :], S_ps, AF.Copy)
                nc.scalar.activation(wgx[:, 1, :], S_ps, AF.Copy, scale=0.0)
                wgy = sm.tile([128, 2, 128], F8, tag="wgy")
                nc.scalar.activation(wgy[:, 0, :], S_ps, AF.Copy, scale=0.0)
                nc.scalar.activation(wgy[:, 1, :], S_ps, AF.Copy)

                nc.tensor.matmul(sretT_ps, Are_s[i][:], stf_bf[:],
                                 start=True, stop=True, skip_group_check=True)
                nc.tensor.matmul(simtT_ps, Aim_s[i][:], stf_bf[:],
                                 start=True, stop=True, skip_group_check=True)
                sret_bf = sm.tile([C, K], BF, tag="sreb")
                nc.scalar.activation(sret_bf[:], sretT_ps, AF.Copy)
                simt_bf = sm.tile([C, K], BF, tag="simb")
                nc.scalar.activation(simt_bf[:], simtT_ps, AF.Copy)
                nc.tensor.matmul(Sre_ps, sret_bf[:], ident_bf[:],
                                 is_transpose=True, skip_group_check=True)
                nc.tensor.matmul(Sim_ps, simt_bf[:], ident_bf[:],
                                 is_transpose=True, skip_group_check=True)
                wbr = sm.tile([128, 2, 128], F8, tag="wbr")
                nc.scalar.activation(wbr[:, 0, :], Sre_ps, AF.Copy)
                nc.scalar.activation(wbr[:, 1, :], Sim_ps, AF.Copy, scale=-1.0)
                wbi = sm.tile([128, 2, 128], F8, tag="wbi")
                nc.scalar.activation(wbi[:, 0, :], Sim_ps, AF.Copy)
                nc.scalar.activation(wbi[:, 1, :], Sre_ps, AF.Copy)

                nc.tensor.matmul(sw0b_ps, stf_bf[:], w0b_s[i][:],
                                 start=True, stop=True, skip_group_check=True)
                sw0b_bf = sm.tile([K, C], BF, tag="sw0b")
                nc.scalar.activation(sw0b_bf[:], sw0b_ps, AF.Copy)

                if i < nb - 1:
                    cc_iA = dram.tile([C, K], F32, tag="ciA")
                    cc_oA = dram.tile([C, K], F32, tag="coA")
                    cc_iB = dram.tile([C, K], F32, tag="ciB")
                    cc_oB = dram.tile([C, K], F32, tag="coB")

                # ---- fused sweep over node chunks, software-pipelined:
                #      stage A(k) = quad + gX/gY evictions + m1/m2/a1,
                #      stage B(k) = tanh + MLP + relus + residual + bridge.
                #      Emitting A(k+1) before B(k) keeps the PE queue from
                #      blocking on the gf dependency. ----
                def stage_1(cI):
                    g_ap = gxy_t[cI][:]
                    gX = pmm.tile([C, CH], F32, tag="mm", name=f"gX{cI}")
                    nc.tensor.matmul(gX[:], wgx[:], g_ap,
                                     start=True, stop=True, perf_mode=DR)
                    gY = pmm.tile([C, CH], F32, tag="mm", name=f"gY{cI}")
                    nc.tensor.matmul(gY[:], wgy[:], g_ap,
                                     start=True, stop=True, perf_mode=DR)
                    gx_sb = wk.tile([C, CH], BF, tag="gxs", name=f"gxs{cI}")
                    nc.scalar.activation(gx_sb[:], gX[:], AF.Copy)
                    gy_sb = wk.tile([C, CH], BF, tag="gys", name=f"gys{cI}")
                    nc.scalar.activation(gy_sb[:], gY[:], AF.Copy)
                    Br = pmm.tile([C, CH], F32, tag="mm", name=f"Br{cI}")
                    nc.tensor.matmul(Br[:], wbr[:], g_ap,
                                     start=True, stop=True, perf_mode=DR)
                    Bi = pmm.tile([C, CH], F32, tag="mm", name=f"Bi{cI}")
                    nc.tensor.matmul(Bi[:], wbi[:], g_ap,
                                     start=True, stop=True, perf_mode=DR)
                    m1 = wk.tile([C, CH], BF, tag="m1", name=f"m1_{cI}")
                    nc.vector.tensor_mul(m1[:], Br[:], gx_sb[:])
                    m2 = wk.tile([C, CH], BF, tag="m2", name=f"m2_{cI}")
                    nc.vector.tensor_mul(m2[:], Bi[:], gy_sb[:])
                    a1 = wk.tile([C, CH], BF, tag="a1", name=f"a1_{cI}", bufs=4)
                    if POOL_A1:
                        nc.gpsimd.tensor_add(a1[:], m1[:], m2[:])
                    else:
                        nc.vector.tensor_add(a1[:], m1[:], m2[:])
                    return a1

                def stage_2(cI):
                    x_ap = trio_t[cI][:, 0, :]
                    ev_ap = trio_t[cI][:, 1, :]
                    h0 = ph0.tile([C, CH], F32, tag="h0", name=f"h0_{cI}")
                    nc.tensor.matmul(h0[:], w0a_s[i][:], x_ap,
                                     start=True, stop=False)
                    nc.tensor.matmul(h0[:], sw0b_bf[:], ev_ap,
                                     start=False, stop=False)
                    return h0

                def stage_3(cI, a1, h0):
                    x_ap = trio_t[cI][:, 0, :]
                    gf = wk.tile([C, CH], BF, tag="gf", name=f"gf{cI}")
                    nc.scalar.activation(gf[:], a1[:], AF.Tanh,
                                         scale=1.0 / (SG * SG))
                    nc.tensor.matmul(h0[:], w0c_s[i][:], gf[:],
                                     start=False, stop=True)
                    h0s = wk.tile([C, CH], BF, tag="h0s", name=f"h0s{cI}")
                    nc.scalar.activation(h0s[:], h0[:], AF.Relu,
                                         bias=b0_s[i][:])
                    h1 = pmm.tile([C, CH], F32, tag="mm", name=f"h1_{cI}")
                    nc.tensor.matmul(h1[:], w1_s[i][:], h0s[:],
                                     start=True, stop=True)
                    h1s = wk.tile([C, CH], BF, tag="h1s", name=f"h1s{cI}")
                    nc.vector.tensor_scalar(h1s[:], h1[:], b1_s[i][:], 0.0,
                                            ALU.add, ALU.max)
                    h2 = pmm.tile([C, CH], F32, tag="mm", name=f"h2_{cI}")
                    nc.tensor.matmul(h2[:], w2_s[i][:], h1s[:],
                                     start=True, stop=True)
                    # x += h2 + b2 (bf16 residual carrier)
                    nc.vector.scalar_tensor_tensor(
                        out=x_ap, in0=h2[:], scalar=b2_s[i][:],
                        in1=x_ap, op0=ALU.add, op1=ALU.add)

                    if i < nb - 1:
                        # bridge: transpose x_new into 4 stacked 128x128
                        # node-major tiles, accumulate spectral delta on PE
                        xnm = wk.tile([128, 4, 128], BF, tag="xnm",
                                      name=f"xnm{cI}")
                        if BRIDGE_XBAR:
                            nc.sync.dma_start_transpose(xnm[:], x_ap)
                        else:
                            hT = pmm.tile([128, 4, 128], BF, tag="mm",
                                          name=f"hT{cI}")
                            for t in range(4):
                                nc.tensor.transpose(
                                    hT[:, t, :],
                                    x_ap[:, t * 128:(t + 1) * 128],
                                    ident_bf[:])
                            nc.vector.tensor_copy(xnm[:], hT[:])
                        spec_dst = specA_ps if cI < nch // 2 else specB_ps
                        first = cI in (0, nch // 2)
                        last = cI in (nch // 2 - 1, nch - 1)
                        for t in range(4):
                            nc.tensor.matmul(
                                spec_dst,
                                xnm[:, t, :],
                                trio_t[cI][:, 2, t * 128:(t + 1) * 128],
                                start=(first and t == 0),
                                stop=(last and t == 3),
                                skip_group_check=True)
                        if cI == nch // 2 - 1:
                            spA = sm.tile([C, K], F32, tag="spA")
                            nc.vector.tensor_copy(spA[:], specA_ps)
                            nc.sync.dma_start(cc_iA[:], spA[:])
                            if USE_CC:
                                nc.gpsimd.collective_compute(
                                    "AllReduce", ALU.add,
                                    replica_groups=PAIRS[:ncores // 2],
                                    ins=[cc_iA.opt()], outs=[cc_oA.opt()])
                            else:
                                nc.sync.dma_start(cc_oA[:], spA[:])
                        elif cI == nch - 1:
                            spB = sm.tile([C, K], F32, tag="spB")
                            nc.vector.tensor_copy(spB[:], specB_ps)
                            nc.sync.dma_start(cc_iB[:], spB[:])
                            if USE_CC:
                                nc.gpsimd.collective_compute(
                                    "AllReduce", ALU.add,
                                    replica_groups=PAIRS[:ncores // 2],
                                    ins=[cc_iB.opt()], outs=[cc_oB.opt()])
                            else:
                                nc.sync.dma_start(cc_oB[:], spB[:])
                    else:
                        # output head
                        y = pmm.tile([3, CH], F32, tag="mm", name=f"y{cI}")
                        nc.tensor.matmul(y[:], wlast_s[:], x_ap,
                                         start=True, stop=True)
                        ysb = wk.tile([3, CH], F32, tag="y", name=f"ys{cI}")
                        nc.vector.tensor_scalar_add(ysb[:], y[:], blast_s[:])
                        nc.sync.dma_start(yT[:, cI * CH:(cI + 1) * CH], ysb[:])

                hist = {}
                for cI in range(nch + 2):
                    if cI < nch:
                        a1_c = stage_1(cI)
                    if 1 <= cI <= nch:
                        h0_c = stage_2(cI - 1)
                        hist[cI - 1] = (hist[cI - 1], h0_c)
                    if cI >= 2:
                        stage_3(cI - 2, *hist.pop(cI - 2))
                    if cI < nch:
                        hist[cI] = a1_c

    nc.compile()
    return nc


_NC_CACHE = {}


def _get_nc():
    if "nc" not in _NC_CACHE:
        _NC_CACHE["nc"] = build_nc()
    return _NC_CACHE["nc"]


def kernel(**inputs):
    nc = _get_nc()
    in_maps = host_prep(inputs)
    res = run_bass_kernel_spmd(nc, in_maps, core_ids=list(range(NCORES)))
    out = np.empty((B, N, 3), np.float32)
    for b in range(B):
        for h in range(2):
            yT = res.results[2 * b + h]["yT"]
            out[b, h * NH:(h + 1) * NH] = yT[:, :NH].T
    return out


# revision 35
# speedup vs baseline: 2.3039x; 1.1681x over previous
"""DiffusionNet forward on 8 Trainium2 NeuronCores.

Strategy (v3)
-------------
B=4 samples, 2 cores per sample, each core owns half the mesh nodes
(20000, zero-padded to 20480).  All cross-node coupling flows through the
K=128 spectral bottleneck:

  * SpMM eliminated on device: gX = (G @ evecs) @ S, host precomputes
    GXe = G @ evecs once per sample (exact associativity).
  * Everything big is SBUF-resident for the whole kernel (no per-block
    re-streaming): x (bf16), ev (bf16, K-major), evm (bf16, node-major)
    and the gradient operators (fp8e4, x64 scaled, [k, j, n] layout with
    j in {gx, gy}).
  * The four spectral-stream matmuls per chunk (gX, gY, Br, Bi) are each
    ONE fp8 DoubleRow matmul contracting 256 = K x {x,y}: the pair dim
    holds the gx/gy interleave, so Br = Sre@gx - Sim@gy needs no PSUM
    accumulation (accumulating DoubleRow pairs crash the runtime).
  * The forward spectral transform of the NEXT block is fused into the
    sweep: after the residual update of a chunk, its x tiles are
    transposed and immediately accumulated into the spectral partial
    (PSUM), split into two halves so the pair AllReduce of the first
    half hides under the second half of the sweep.
  * Block 1's spectrum is precomputed on host (full-sample sum), so no
    standalone forward pass and no AllReduce before the first sweep.
  * Elementwise work is spread over DVE (m1, m2, residual), Act (Br/Bi
    evictions, tanh, relus, casts) and Pool/gpsimd (a1 = m1 + m2).
"""

import sys
import numpy as np
import ml_dtypes

for _p in ("/opt/trn_rl_repo", "/root/.axon_site/_ro/trn_rl_repo"):
    if _p not in sys.path:
        sys.path.append(_p)

import concourse.bass as bass
import concourse.bacc as bacc
import concourse.tile as tile
import concourse.mybir as mybir
from concourse.bass_utils import run_bass_kernel_spmd
from concourse.masks import make_identity

BF = mybir.dt.bfloat16
F32 = mybir.dt.float32
F8 = mybir.dt.float8e4
AF = mybir.ActivationFunctionType
ALU = mybir.AluOpType
DR = mybir.MatmulPerfMode.DoubleRow

B, N, E, K = 4, 40000, 240000, 128
C = 128
NB = 4          # diffusion blocks
NCORES = 8
NH = N // 2     # nodes per core (half sample)
CH = 512        # node chunk (matmul free dim)
NHP = 20480     # padded nodes per core: 40 chunks * 512
NCH = NHP // CH
PAIRS = [[0, 1], [2, 3], [4, 5], [6, 7]]
SG = 64.0       # fp8 scale on GXe/GYe; tanh un-scales by 1/SG^2
BRIDGE_XBAR = True   # xbar-DMA transpose bridge vs PE transpose + DVE evict
POOL_A1 = True       # a1 on gpsimd/Pool vs DVE
USE_CC = True        # pairwise AllReduce vs local-only (debug)

bf16 = ml_dtypes.bfloat16
f8e4 = ml_dtypes.float8_e4m3


# ----------------------------------------------------------------- host side

def _spmm_mat(rows, cols, vals, M):
    """(COO [N,N] with given pattern) @ M, dense M [N,k]. Pure numpy."""
    out = np.zeros((N, M.shape[1]), np.float32)
    perm = np.argsort(rows, kind="stable")
    contrib = (vals[:, None] * M[cols]).astype(np.float32)[perm]
    rs = rows[perm]
    uniq, starts = np.unique(rs, return_index=True)
    out[uniq] = np.add.reduceat(contrib, starts, axis=0)
    return out


def host_prep(inputs, nhp=NHP, nb=NB):
    """Build the 8 per-core input dicts."""
    x_in = np.asarray(inputs["x_in"], np.float32)
    mass = np.asarray(inputs["mass"], np.float32)
    evals = np.asarray(inputs["evals"], np.float32)
    evecs = np.asarray(inputs["evecs"], np.float32)
    rows = np.asarray(inputs["rows"])
    cols = np.asarray(inputs["cols"])
    gX_vals = np.asarray(inputs["gradX_vals"], np.float32)
    gY_vals = np.asarray(inputs["gradY_vals"], np.float32)
    w_first = np.asarray(inputs["w_first"], np.float32)
    b_first = np.asarray(inputs["b_first"], np.float32)
    diff_time = np.asarray(inputs["diff_time"], np.float32)
    A_re = np.asarray(inputs["A_re"], np.float32)
    A_im = np.asarray(inputs["A_im"], np.float32)
    mlp_w0 = np.asarray(inputs["mlp_w0"], np.float32)
    w1 = np.asarray(inputs["mlp_w1"], np.float32)
    w2 = np.asarray(inputs["mlp_w2"], np.float32)
    b0 = np.asarray(inputs["mlp_b0"], np.float32)
    b1 = np.asarray(inputs["mlp_b1"], np.float32)
    b2 = np.asarray(inputs["mlp_b2"], np.float32)
    w_last = np.asarray(inputs["w_last"], np.float32)
    b_last = np.asarray(inputs["b_last"], np.float32)

    nh = NH

    shared = dict(
        Are=A_re[:nb].astype(bf16),
        Aim=A_im[:nb].astype(bf16),
        w0a=np.ascontiguousarray(mlp_w0[:nb, 0:C]).astype(bf16),
        w0b=np.ascontiguousarray(mlp_w0[:nb, C:2 * C]).astype(bf16),
        w0c=np.ascontiguousarray(mlp_w0[:nb, 2 * C:3 * C]).astype(bf16),
        w1=w1[:nb].astype(bf16),
        w2=w2[:nb].astype(bf16),
        b0=b0[:nb].reshape(nb, C, 1),
        b1=b1[:nb].reshape(nb, C, 1),
        b2=b2[:nb].reshape(nb, C, 1),
        wlast=w_last.astype(bf16),
        blast=b_last.reshape(3, 1),
    )

    in_maps = []
    for b in range(B):
        ev = evecs[b]
        evm_full = ev * mass[b][:, None]
        GXe = _spmm_mat(rows, cols, gX_vals[b], ev)
        GYe = _spmm_mat(rows, cols, gY_vals[b], ev)
        x0_full = x_in[b] @ w_first + b_first
        # spec for block 0, full-sample sum (both halves): [C, K]
        spec1 = (x0_full.T @ evm_full).astype(np.float32)
        # coefs[i][c,k] = exp(-evals[k] * diff_time[i][c])
        coefs = np.exp(-evals[b][None, None, :]
                       * diff_time[:nb, :, None]).astype(np.float32)
        for h in range(2):
            sl = slice(h * nh, (h + 1) * nh)

            def padT(M):  # [nh, D] -> [D, nhp]
                out = np.zeros((M.shape[1], nhp), np.float32)
                out[:, :nh] = M[sl].T
                return out

            # trio [nch, 128, 3, 512] bf16: 0 = x0, 1 = ev (K-major),
            # 2 = evm node-major tiles (t, k)
            x0c = padT(x0_full).reshape(C, NCH, CH).transpose(1, 0, 2)
            evc = padT(ev).reshape(K, NCH, CH).transpose(1, 0, 2)
            evmP = np.zeros((nhp, K), np.float32)
            evmP[:nh] = evm_full[sl]
            evmc = evmP.reshape(NCH, 4, 128, K).transpose(0, 2, 1, 3) \
                       .reshape(NCH, 128, CH)
            trio = np.stack([x0c, evc, evmc], axis=2).astype(bf16)

            # gxy [nch, 128(k), 2(j), 512(n)] fp8: j=0 gx, j=1 gy
            def kpack(Gm):  # [nh, K] -> [nch, 128, 512]
                GT = np.clip(padT(Gm) * SG, -240.0, 240.0)  # [K, nhp]
                return GT.reshape(K, NCH, CH).transpose(1, 0, 2)

            gxy = np.stack([kpack(GXe), kpack(GYe)], axis=2).astype(f8e4)

            in_maps.append(dict(
                trio=trio,
                gxy=np.ascontiguousarray(gxy),
                spec1=spec1,
                coefs=coefs,
                **shared,
            ))
    return in_maps


# --------------------------------------------------------------- device side

def build_nc(nb=NB, nch=NCH, ncores=NCORES):
    nhp = nch * CH
    nc = bacc.Bacc("TRN2", target_bir_lowering=False, debug=False,
                   enable_asserts=False, num_devices=ncores)

    trio = nc.dram_tensor("trio", [nch, 128, 3, CH], BF, kind="ExternalInput")
    gxy = nc.dram_tensor("gxy", [nch, 128, 2, CH], F8, kind="ExternalInput")
    spec1 = nc.dram_tensor("spec1", [C, K], F32, kind="ExternalInput")
    coefs = nc.dram_tensor("coefs", [nb, C, K], F32, kind="ExternalInput")
    Are = nc.dram_tensor("Are", [nb, C, C], BF, kind="ExternalInput")
    Aim = nc.dram_tensor("Aim", [nb, C, C], BF, kind="ExternalInput")
    w0a = nc.dram_tensor("w0a", [nb, C, C], BF, kind="ExternalInput")
    w0b = nc.dram_tensor("w0b", [nb, C, C], BF, kind="ExternalInput")
    w0c = nc.dram_tensor("w0c", [nb, C, C], BF, kind="ExternalInput")
    w1 = nc.dram_tensor("w1", [nb, C, C], BF, kind="ExternalInput")
    w2 = nc.dram_tensor("w2", [nb, C, C], BF, kind="ExternalInput")
    b0 = nc.dram_tensor("b0", [nb, C, 1], F32, kind="ExternalInput")
    b1 = nc.dram_tensor("b1", [nb, C, 1], F32, kind="ExternalInput")
    b2 = nc.dram_tensor("b2", [nb, C, 1], F32, kind="ExternalInput")
    wlast = nc.dram_tensor("wlast", [C, 3], BF, kind="ExternalInput")
    blast = nc.dram_tensor("blast", [3, 1], F32, kind="ExternalInput")
    yT = nc.dram_tensor("yT", [3, nhp], F32, kind="ExternalOutput")

    with tile.TileContext(nc) as tc:
        with (
            tc.tile_pool(name="consts", bufs=1) as consts,
            tc.tile_pool(name="res", bufs=1) as res,
            tc.tile_pool(name="wk", bufs=2) as wk,
            tc.tile_pool(name="sm", bufs=2) as sm,
            tc.tile_pool(name="pmm", bufs=5, space="PSUM") as pmm,
            tc.tile_pool(name="ph0", bufs=2, space="PSUM") as ph0,
            tc.tile_pool(name="psm", bufs=1, space="PSUM") as psm,
            tc.tile_pool(name="dram", bufs=2, space="DRAM") as dram,
        ):
            ident_bf = consts.tile([128, 128], BF, tag="identb")
            make_identity(nc, ident_bf[:])

            def cload(src, shape, dt, tag):
                t = consts.tile(shape, dt, tag=tag)
                nc.sync.dma_start(t[:], src)
                return t

            coefs_s = [cload(coefs[i], [C, K], F32, f"cf{i}") for i in range(nb)]
            Are_s = [cload(Are[i], [C, C], BF, f"Are{i}") for i in range(nb)]
            Aim_s = [cload(Aim[i], [C, C], BF, f"Aim{i}") for i in range(nb)]
            w0a_s = [cload(w0a[i], [C, C], BF, f"w0a{i}") for i in range(nb)]
            w0b_s = [cload(w0b[i], [C, C], BF, f"w0b{i}") for i in range(nb)]
            w0c_s = [cload(w0c[i], [C, C], BF, f"w0c{i}") for i in range(nb)]
            w1_s = [cload(w1[i], [C, C], BF, f"w1{i}") for i in range(nb)]
            w2_s = [cload(w2[i], [C, C], BF, f"w2{i}") for i in range(nb)]
            b0_s = [cload(b0[i], [C, 1], F32, f"b0{i}") for i in range(nb)]
            b1_s = [cload(b1[i], [C, 1], F32, f"b1{i}") for i in range(nb)]
            b2_s = [cload(b2[i], [C, 1], F32, f"b2{i}") for i in range(nb)]
            wlast_s = cload(wlast[:], [C, 3], BF, "wlast")
            blast_s = cload(blast[:], [3, 1], F32, "blast")
            spec1_s = cload(spec1[:], [C, K], F32, "spec1")

            # resident per-chunk tiles, streamed in once
            trio_t, gxy_t = [], []
            for cI in range(nch):
                t = res.tile([128, 3, CH], BF, tag=f"trio{cI}")
                nc.sync.dma_start(t[:], trio[cI])
                trio_t.append(t)
                g = res.tile([128, 2, CH], F8, tag=f"gxy{cI}")
                nc.sync.dma_start(g[:], gxy[cI])
                gxy_t.append(g)

            # PSUM scratch banks for small accumulators, manually packed
            s1 = psm.tile([128, CH], F32, tag="s1")
            specA_ps = s1[:, 0:128]
            specB_ps = s1[:, 128:256]
            sw0b_ps = s1[:, 256:384]                      # [K, C] f32

            if USE_CC:
                # collective warmup (ring spin-up off the critical path)
                warm_sb = sm.tile([C, K], F32, tag="warm", bufs=1)
                nc.gpsimd.memset(warm_sb[:], 0.0)
                cc_wi = dram.tile([C, K], F32, tag="cwi", bufs=1)
                cc_wo = dram.tile([C, K], F32, tag="cwo", bufs=1)
                nc.sync.dma_start(cc_wi[:], warm_sb[:])
                nc.gpsimd.collective_compute(
                    "AllReduce", ALU.add, replica_groups=PAIRS[:ncores // 2],
                    ins=[cc_wi.opt()], outs=[cc_wo.opt()])

            for i in range(nb):
                # ---- smalls: DoubleRow W matrices from the block spectrum
                #      WgX=[S|0] WgY=[0|S] WBr=[Sre|-Sim] WBi=[Sim|Sre],
                #      each [k, j, c] fp8 with j the gx/gy pair dim ----
                if i == 0:
                    spec_f = spec1_s
                else:
                    sA = sm.tile([C, K], F32, tag="sA")
                    nc.sync.dma_start(sA[:], cc_oA[:])
                    sB = sm.tile([C, K], F32, tag="sB")
                    nc.sync.dma_start(sB[:], cc_oB[:])
                    spec_f = sm.tile([C, K], F32, tag="spec")
                    nc.vector.tensor_add(spec_f[:], sA[:], sB[:])

                s2 = pmm.tile([128, CH], F32, tag="mm", name=f"sml{i}")
                sretT_ps = s2[:, 0:128]                   # Sre^T [C, K] f32
                simtT_ps = s2[:, 128:256]                 # Sim^T [C, K] f32
                bfh = s2[:, 256:384].bitcast(BF)          # [128, 256] bf16
                S_ps = bfh[:, 0:128]                      # S [K, C] bf16
                Sre_ps = bfh[:, 128:256]                  # Sre [K, C] bf16
                Sim_ps = s2[:, 384:448].bitcast(BF)       # Sim [K, C] bf16
                stf_bf = sm.tile([C, K], BF, tag="stfb")
                nc.vector.tensor_mul(stf_bf[:], spec_f[:], coefs_s[i][:])
                # S = (S^T)^T first: WgX/WgY unblock the sweep
                nc.tensor.matmul(S_ps, stf_bf[:], ident_bf[:],
                                 is_transpose=True, skip_group_check=True)
                wgx = sm.tile([128, 2, 128], F8, tag="wgx")
                nc.scalar.activation(wgx[:, 0, :], S_ps, AF.Copy)
                nc.scalar.activation(wgx[:, 1, :], S_ps, AF.Copy, scale=0.0)
                wgy = sm.tile([128, 2, 128], F8, tag="wgy")
                nc.scalar.activation(wgy[:, 0, :], S_ps, AF.Copy, scale=0.0)
                nc.scalar.activation(wgy[:, 1, :], S_ps, AF.Copy)

                nc.tensor.matmul(sretT_ps, Are_s[i][:], stf_bf[:],
                                 start=True, stop=True, skip_group_check=True)
                nc.tensor.matmul(simtT_ps, Aim_s[i][:], stf_bf[:],
                                 start=True, stop=True, skip_group_check=True)
                sret_bf = sm.tile([C, K], BF, tag="sreb")
                nc.scalar.activation(sret_bf[:], sretT_ps, AF.Copy)
                simt_bf = sm.tile([C, K], BF, tag="simb")
                nc.scalar.activation(simt_bf[:], simtT_ps, AF.Copy)
                nc.tensor.matmul(Sre_ps, sret_bf[:], ident_bf[:],
                                 is_transpose=True, skip_group_check=True)
                nc.tensor.matmul(Sim_ps, simt_bf[:], ident_bf[:],
                                 is_transpose=True, skip_group_check=True)
                wbr = sm.tile([128, 2, 128], F8, tag="wbr")
                nc.scalar.activation(wbr[:, 0, :], Sre_ps, AF.Copy)
                nc.scalar.activation(wbr[:, 1, :], Sim_ps, AF.Copy, scale=-1.0)
                wbi = sm.tile([128, 2, 128], F8, tag="wbi")
                nc.scalar.activation(wbi[:, 0, :], Sim_ps, AF.Copy)
                nc.scalar.activation(wbi[:, 1, :], Sre_ps, AF.Copy)

                nc.tensor.matmul(sw0b_ps, stf_bf[:], w0b_s[i][:],
                                 start=True, stop=True, skip_group_check=True)
                sw0b_bf = sm.tile([K, C], BF, tag="sw0b")
                nc.scalar.activation(sw0b_bf[:], sw0b_ps, AF.Copy)

                if i < nb - 1:
                    cc_iA = dram.tile([C, K], F32, tag="ciA")
                    cc_oA = dram.tile([C, K], F32, tag="coA")
                    cc_iB = dram.tile([C, K], F32, tag="ciB")
                    cc_oB = dram.tile([C, K], F32, tag="coB")

                # ---- fused sweep over node chunks, software-pipelined:
                #      stage A(k) = quad + gX/gY evictions + m1/m2/a1,
                #      stage B(k) = tanh + MLP + relus + residual + bridge.
                #      Emitting A(k+1) before B(k) keeps the PE queue from
                #      blocking on the gf dependency. ----
                def stage_1(cI):
                    g_ap = gxy_t[cI][:]
                    gX = pmm.tile([C, CH], F32, tag="mm", name=f"gX{cI}")
                    nc.tensor.matmul(gX[:], wgx[:], g_ap,
                                     start=True, stop=True, perf_mode=DR)
                    gY = pmm.tile([C, CH], F32, tag="mm", name=f"gY{cI}")
                    nc.tensor.matmul(gY[:], wgy[:], g_ap,
                                     start=True, stop=True, perf_mode=DR)
                    gx_sb = wk.tile([C, CH], BF, tag="gxs", name=f"gxs{cI}")
                    nc.scalar.activation(gx_sb[:], gX[:], AF.Copy)
                    gy_sb = wk.tile([C, CH], BF, tag="gys", name=f"gys{cI}")
                    nc.scalar.activation(gy_sb[:], gY[:], AF.Copy)
                    Br = pmm.tile([C, CH], F32, tag="mm", name=f"Br{cI}")
                    nc.tensor.matmul(Br[:], wbr[:], g_ap,
                                     start=True, stop=True, perf_mode=DR)
                    Bi = pmm.tile([C, CH], F32, tag="mm", name=f"Bi{cI}")
                    nc.tensor.matmul(Bi[:], wbi[:], g_ap,
                                     start=True, stop=True, perf_mode=DR)
                    m1 = wk.tile([C, CH], BF, tag="m1", name=f"m1_{cI}")
                    nc.vector.tensor_mul(m1[:], Br[:], gx_sb[:])
                    m2 = wk.tile([C, CH], BF, tag="m2", name=f"m2_{cI}")
                    nc.vector.tensor_mul(m2[:], Bi[:], gy_sb[:])
                    a1 = wk.tile([C, CH], BF, tag="a1", name=f"a1_{cI}",
                                 bufs=4)
                    if POOL_A1:
                        nc.gpsimd.tensor_add(a1[:], m1[:], m2[:])
                    else:
                        nc.vector.tensor_add(a1[:], m1[:], m2[:])
                    return a1

                def stage_2(cI):
                    h0 = ph0.tile([C, CH], F32, tag="h0", name=f"h0_{cI}")
                    nc.tensor.matmul(h0[:], w0a_s[i][:], trio_t[cI][:, 0, :],
                                     start=True, stop=False)
                    nc.tensor.matmul(h0[:], sw0b_bf[:], trio_t[cI][:, 1, :],
                                     start=False, stop=False)
                    return h0

                def stage_3a(cI, a1, h0):
                    gf = wk.tile([C, CH], BF, tag="gf", name=f"gf{cI}")
                    nc.scalar.activation(gf[:], a1[:], AF.Tanh,
                                         scale=1.0 / (SG * SG))
                    nc.tensor.matmul(h0[:], w0c_s[i][:], gf[:],
                                     start=False, stop=True)
                    h0s = wk.tile([C, CH], BF, tag="h0s", name=f"h0s{cI}")
                    nc.scalar.activation(h0s[:], h0[:], AF.Relu,
                                         bias=b0_s[i][:])
                    return h0s

                def stage_3b(cI, h0s):
                    h1 = pmm.tile([C, CH], F32, tag="mm", name=f"h1_{cI}")
                    nc.tensor.matmul(h1[:], w1_s[i][:], h0s[:],
                                     start=True, stop=True)
                    h1s = wk.tile([C, CH], BF, tag="h1s", name=f"h1s{cI}")
                    nc.vector.tensor_scalar(h1s[:], h1[:], b1_s[i][:], 0.0,
                                            ALU.add, ALU.max)
                    return h1s

                def stage_3c(cI, h1s):
                    x_ap = trio_t[cI][:, 0, :]
                    h2 = pmm.tile([C, CH], F32, tag="mm", name=f"h2_{cI}")
                    nc.tensor.matmul(h2[:], w2_s[i][:], h1s[:],
                                     start=True, stop=True)
                    # x += h2 + b2 (bf16 residual carrier)
                    nc.vector.scalar_tensor_tensor(
                        out=x_ap, in0=h2[:], scalar=b2_s[i][:],
                        in1=x_ap, op0=ALU.add, op1=ALU.add)
                    if i < nb - 1:
                        xnm = wk.tile([128, 4, 128], BF, tag="xnm",
                                      name=f"xnm{cI}")
                        if BRIDGE_XBAR:
                            nc.sync.dma_start_transpose(xnm[:], x_ap)
                        else:
                            hT = pmm.tile([128, 4, 128], BF, tag="mm",
                                          name=f"hT{cI}")
                            for t in range(4):
                                nc.tensor.transpose(
                                    hT[:, t, :],
                                    x_ap[:, t * 128:(t + 1) * 128],
                                    ident_bf[:])
                            nc.vector.tensor_copy(xnm[:], hT[:])
                        return xnm
                    # output head
                    y = pmm.tile([3, CH], F32, tag="mm", name=f"y{cI}")
                    nc.tensor.matmul(y[:], wlast_s[:], x_ap,
                                     start=True, stop=True)
                    ysb = wk.tile([3, CH], F32, tag="y", name=f"ys{cI}")
                    nc.vector.tensor_scalar_add(ysb[:], y[:], blast_s[:])
                    nc.sync.dma_start(yT[:, cI * CH:(cI + 1) * CH], ysb[:])
                    return None

                def stage_3d(cI, xnm):
                    if i >= nb - 1:
                        return
                    spec_dst = specA_ps if cI < nch // 2 else specB_ps
                    first = cI in (0, nch // 2)
                    last = cI in (nch // 2 - 1, nch - 1)
                    for t in range(4):
                        nc.tensor.matmul(
                            spec_dst,
                            xnm[:, t, :],
                            trio_t[cI][:, 2, t * 128:(t + 1) * 128],
                            start=(first and t == 0),
                            stop=(last and t == 3),
                            skip_group_check=True)
                    if cI == nch // 2 - 1:
                        spA = sm.tile([C, K], F32, tag="spA")
                        nc.vector.tensor_copy(spA[:], specA_ps)
                        nc.sync.dma_start(cc_iA[:], spA[:])
                        if USE_CC:
                            nc.gpsimd.collective_compute(
                                "AllReduce", ALU.add,
                                replica_groups=PAIRS[:ncores // 2],
                                ins=[cc_iA.opt()], outs=[cc_oA.opt()])
                        else:
                            nc.sync.dma_start(cc_oA[:], spA[:])
                    elif cI == nch - 1:
                        spB = sm.tile([C, K], F32, tag="spB")
                        nc.vector.tensor_copy(spB[:], specB_ps)
                        nc.sync.dma_start(cc_iB[:], spB[:])
                        if USE_CC:
                            nc.gpsimd.collective_compute(
                                "AllReduce", ALU.add,
                                replica_groups=PAIRS[:ncores // 2],
                                ins=[cc_iB.opt()], outs=[cc_oB.opt()])
                        else:
                            nc.sync.dma_start(cc_oB[:], spB[:])

                # 6-deep software pipeline over the chunk loop: every PE
                # instruction's cross-engine input is >= 1 iteration old.
                vals = {}
                for t in range(nch + 5):
                    if t < nch:
                        vals[(1, t)] = stage_1(t)
                    if 0 <= t - 1 < nch:
                        vals[(2, t - 1)] = stage_2(t - 1)
                    if 0 <= t - 2 < nch:
                        c = t - 2
                        vals[(3, c)] = stage_3a(c, vals.pop((1, c)),
                                                vals.pop((2, c)))
                    if 0 <= t - 3 < nch:
                        c = t - 3
                        vals[(4, c)] = stage_3b(c, vals.pop((3, c)))
                    if 0 <= t - 4 < nch:
                        c = t - 4
                        vals[(5, c)] = stage_3c(c, vals.pop((4, c)))
                    if 0 <= t - 5 < nch:
                        c = t - 5
                        stage_3d(c, vals.pop((5, c)))

    nc.compile()
    return nc


_NC_CACHE = {}


def _get_nc():
    if "nc" not in _NC_CACHE:
        _NC_CACHE["nc"] = build_nc()
    return _NC_CACHE["nc"]


def kernel(**inputs):
    nc = _get_nc()
    in_maps = host_prep(inputs)
    res = run_bass_kernel_spmd(nc, in_maps, core_ids=list(range(NCORES)))
    out = np.empty((B, N, 3), np.float32)
    for b in range(B):
        for h in range(2):
            yT = res.results[2 * b + h]["yT"]
            out[b, h * NH:(h + 1) * NH] = yT[:, :NH].T
    return out
